# revision 11
# baseline (speedup 1.0000x reference)
"""Trainium2 Bass kernel for GCN(3-layer) + BiLSTM(2-layer) + FC.

Self-contained: hardcodes all shapes; needs /opt/trn_rl_repo (concourse) only.

Architecture (8 NeuronCores, SPMD):
 - Nodes sharded by dst range (40960/core, graph-aligned).  A_hat = D^-1/2 (A+I) D^-1/2.
 - GCN layer 1: src features pre-gathered on host per (window, cell, slot) ->
   contiguous HWDGE loads, no indirect DMA.
 - GCN layers 2/3: ONE batched indirect gather per 512-node window (idx block
   [128, 43]), scatter-add via PE matmuls against host-built sparse cell
   blocks S (norm weights folded in), fused bias/relu on ACT, next-layer
   premultiply m_{k+1} = h_k @ W on PE, AllGather of m shards between layers.
 - BiLSTM: batch-sharded 16 seq/core, feature-major state (H=128 partitions),
   per-step gate matmuls on PE, sigmoid/tanh on ACT, cell math on DVE.
 - All contiguous DMA on SP HWDGE; GpSimd (SWDGE) reserved for the two
   indirect gathers per layer-window and collectives.
"""
import sys, os
sys.path.insert(0, "/opt/trn_rl_repo")
import numpy as np
import ml_dtypes

import concourse.bass as bass
import concourse.bacc as bacc
import concourse.tile as tile
from concourse import mybir
from concourse.bass_utils import run_bass_kernel_spmd

BF16 = ml_dtypes.bfloat16
AF = mybir.ActivationFunctionType

# ---- problem constants (hardcoded) ----
B, T, FEAT, H, GCN_H, NCLS = 128, 512, 320, 128, 32, 10
N = B * 5 * T            # 327680
FIN = FEAT // 5          # 64
LSTM_IN = 5 * GCN_H      # 160
NCORES = 8
NS = N // NCORES         # 40960 nodes/core
WIN = 512                # window cols
NW = NS // WIN           # 80 windows/core
CELL = 16
NCELL = 32               # regular cells per window (16 cols each)
NCT = NCELL + 1          # + 1 spill cell (full-window S, accumulated last)
MAXE = 128               # max edges per cell slot-block
BC = B // NCORES         # 16 sequences per core
TB = T * BC              # 8192 token-cols per core
XWCH = 16                # lstm xw prefetch chunk (steps)


# =====================================================================
# Host preprocessing
# =====================================================================

def _prep_graph(x, edge_src, edge_dst):
    src = np.asarray(edge_src, np.int64)
    dst = np.asarray(edge_dst, np.int64)
    deg = np.bincount(dst, minlength=N).astype(np.float64) + 1.0
    dinv = 1.0 / np.sqrt(deg)
    sl = np.arange(N, dtype=np.int64)
    s_all = np.concatenate([src, sl])
    d_all = np.concatenate([dst, sl])
    w_all = (dinv[s_all] * dinv[d_all]).astype(np.float32)
    x_bf = np.asarray(x, np.float32).astype(BF16)

    # layers 2/3 gather-index remap: m rows are stored (window, p, c) packed
    vv = np.arange(N, dtype=np.int64)
    loc = vv % WIN
    M_REMAP = (vv // WIN) * WIN + (loc % 128) * 4 + loc // 128
    xg_cores, idx_cores, S_cores = [], [], []
    for c in range(NCORES):
        m = (d_all // NS) == c
        s_c = s_all[m]; d_c = d_all[m] - c * NS; w_c = w_all[m]
        wid = d_c // WIN
        cj = (d_c % WIN) // CELL
        cell_id = wid * NCELL + cj
        order = np.argsort(cell_id, kind="stable")
        s_c = s_c[order]; d_c = d_c[order]; w_c = w_c[order]
        cell_id = cell_id[order]
        counts = np.bincount(cell_id, minlength=NW * NCELL)
        starts = np.concatenate([[0], np.cumsum(counts)])
        rank = np.arange(len(s_c)) - starts[cell_id]
        wi = cell_id // NCELL
        ji = cell_id % NCELL
        reg = rank < MAXE
        # spill: per-window running slot for overflow edges
        sp = ~reg
        spill_slot = np.zeros(len(s_c), np.int64)
        ww = wi[sp]
        srt = np.argsort(ww, kind="stable")
        sslot = np.zeros(sp.sum(), np.int64)
        wcounts = np.bincount(ww, minlength=NW)
        wstarts = np.concatenate([[0], np.cumsum(wcounts)])
        sslot[srt] = np.arange(sp.sum()) - wstarts[ww[srt]]
        assert sp.sum() == 0 or sslot.max() < MAXE
        idx_arr = np.zeros((NW, NCT, MAXE), np.int32)
        idx_arr[wi[reg], ji[reg], rank[reg]] = s_c[reg]
        idx_arr[wi[sp], NCELL, sslot] = s_c[sp]
        # regular S: (MAXE, NW*512): cell j owns cols [w*512+j*16, +16)
        S_dev = np.zeros((MAXE, NW * WIN), np.float32)
        S_dev[rank[reg], wi[reg] * WIN + (d_c[reg] % WIN)] = w_c[reg]
        # spill S: (MAXE, NW*512) full-window routing for spill slots
        Ssp = np.zeros((MAXE, NW * WIN), np.float32)
        Ssp[sslot, wi[sp] * WIN + (d_c[sp] % WIN)] = w_c[sp]
        # layer-1 host pre-gather in (w, slot, cell) blocks
        xg = np.ascontiguousarray(
            x_bf[idx_arr].transpose(0, 2, 1, 3).reshape(NW, MAXE, NCT * FIN))
        xg_cores.append(xg)
        i1 = np.ascontiguousarray(
            idx_arr.transpose(2, 0, 1).reshape(MAXE, NW * NCT))
        i23 = M_REMAP[i1].astype(np.int32)
        idx_cores.append(i23)
        S_cores.append((S_dev.astype(BF16), Ssp.astype(BF16)))
    return xg_cores, idx_cores, S_cores


def _prep_weights(inp):
    d = {}
    d["W1"] = np.asarray(inp["W1"], np.float32).astype(BF16)
    d["W2"] = np.asarray(inp["W2"], np.float32).astype(BF16)
    d["W3"] = np.asarray(inp["W3"], np.float32).astype(BF16)
    for k in ("b1", "b2", "b3"):
        d[k] = np.asarray(inp[k], np.float32).reshape(GCN_H, 1)

    # sigmoid(x) computed as 0.5 + 0.5*tanh(x/2): fold the 1/2 into the
    # i, f, o gate weights+bias (torch gate order i,f,g,o -> scale 0,1,3)
    GSCL = np.repeat([0.5, 0.5, 1.0, 0.5], H).astype(np.float32)

    def pack_ih(Wih, bih, bhh):
        Wih = np.asarray(Wih, np.float32)
        inn = Wih.shape[1]
        o = np.zeros((inn + 1, 4 * H), np.float32)
        o[:inn] = Wih.T
        o[inn] = np.asarray(bih, np.float32) + np.asarray(bhh, np.float32)
        return (o * GSCL).astype(BF16)

    def pack_hh(Whh):
        Whh = np.asarray(Whh, np.float32)
        return (np.concatenate(
            [Whh[g*H:(g+1)*H, :].T for g in range(4)], axis=1) * GSCL).astype(BF16)

    for tag in ("0", "1"):
        for dr in ("f", "b"):
            d[f"wih{tag}{dr}"] = pack_ih(
                inp[f"Wih{tag}{dr}"], inp[f"bih{tag}{dr}"], inp[f"bhh{tag}{dr}"])
            d[f"whh{tag}{dr}"] = pack_hh(inp[f"Whh{tag}{dr}"])
    d["wfc"] = np.asarray(inp["Wfc"], np.float32).astype(BF16)
    d["bfc"] = np.broadcast_to(
        np.asarray(inp["bfc"], np.float32), (BC, NCLS)).copy()
    return d


# =====================================================================
# Bass kernel builder
# =====================================================================

# torch gate order i,f,g,o -> device col blocks [i f o g]
GBLK = {0: 0, 1: 1, 2: 3, 3: 2}
USE_IDMM = os.environ.get("KIDMM", "0") == "1"
USE_GPS = os.environ.get("KGPS", "1") == "1"


def build_kernel():
    nc = bacc.Bacc(None, num_devices=NCORES)
    dt = mybir.dt
    f32, bf16, i32 = dt.float32, dt.bfloat16, dt.int32

    xg = nc.dram_tensor("xg", [NW, MAXE, NCT * FIN], bf16, kind="ExternalInput")
    idxT2 = nc.dram_tensor("idxT2", [MAXE, NW * NCT], i32, kind="ExternalInput")
    identT = nc.dram_tensor("identT", [128, 128], bf16, kind="ExternalInput")
    ST = nc.dram_tensor("ST", [MAXE, NW * WIN], bf16, kind="ExternalInput")
    SSP = nc.dram_tensor("SSP", [MAXE, NW * WIN], bf16, kind="ExternalInput")
    W1 = nc.dram_tensor("W1", [FIN, GCN_H], bf16, kind="ExternalInput")
    W2 = nc.dram_tensor("W2", [GCN_H, GCN_H], bf16, kind="ExternalInput")
    W3 = nc.dram_tensor("W3", [GCN_H, GCN_H], bf16, kind="ExternalInput")
    b1 = nc.dram_tensor("b1", [GCN_H, 1], f32, kind="ExternalInput")
    b2 = nc.dram_tensor("b2", [GCN_H, 1], f32, kind="ExternalInput")
    b3 = nc.dram_tensor("b3", [GCN_H, 1], f32, kind="ExternalInput")
    wih, whh = {}, {}
    for tag, inn in (("0", LSTM_IN), ("1", 2 * H)):
        for dr in ("f", "b"):
            wih[tag+dr] = nc.dram_tensor(f"wih{tag}{dr}", [inn + 1, 4 * H], bf16, kind="ExternalInput")
            whh[tag+dr] = nc.dram_tensor(f"whh{tag}{dr}", [H, 4 * H], bf16, kind="ExternalInput")
    wfc = nc.dram_tensor("wfc", [2 * H, NCLS], bf16, kind="ExternalInput")
    bfc = nc.dram_tensor("bfc", [BC, NCLS], f32, kind="ExternalInput")
    out = nc.dram_tensor("out", [BC, NCLS], f32, kind="ExternalOutput")

    dbg = os.environ.get("KDEBUG", "0") == "1"
    m2s = nc.dram_tensor("m2s", [NS, GCN_H], bf16)
    m3s = nc.dram_tensor("m3s", [NS, GCN_H], bf16)
    if dbg:
        m2d = nc.dram_tensor("m2d", [NS, GCN_H], bf16, kind="ExternalOutput")
        m3d = nc.dram_tensor("m3d", [NS, GCN_H], bf16, kind="ExternalOutput")
        gcn3d = nc.dram_tensor("gcn3d", [GCN_H, 5 * TB], bf16, kind="ExternalOutput")
        h0d = nc.dram_tensor("h0d", [H, 2 * TB], bf16, kind="ExternalOutput")
    m2f = nc.dram_tensor("m2f", [N, GCN_H], bf16, addr_space="Shared")
    m3f = nc.dram_tensor("m3f", [N, GCN_H], bf16, addr_space="Shared")
    # xw tensors: per phase, per dir: (H, T*4*BC): col = t*64 + blk*16 + b
    xwd = {ph+dr: nc.dram_tensor(f"xw{ph}{dr}", [H, T * 4 * BC], bf16)
           for ph in ("0", "1") for dr in ("f", "b")}

    RG = [list(range(NCORES))]

    with tile.TileContext(nc) as tc:
        with tc.tile_pool(name="persist", bufs=1) as pp:
            w1_sb = pp.tile([FIN, GCN_H], bf16)
            nc.sync.dma_start(w1_sb[:], W1[:])
            w2_sb = pp.tile([GCN_H, GCN_H], bf16)
            nc.sync.dma_start(w2_sb[:], W2[:])
            w3_sb = pp.tile([GCN_H, GCN_H], bf16)
            nc.sync.dma_start(w3_sb[:], W3[:])
            b_sb = {}
            for nm, t_ in (("b1", b1), ("b2", b2), ("b3", b3)):
                bias_tile = pp.tile([GCN_H, 1], f32, tag=nm)
                b_sb[nm] = bias_tile
                nc.sync.dma_start(b_sb[nm][:], t_[:])
            tokAll = pp.tile([GCN_H, 5 * TB], bf16)

            # ---------------- GCN ----------------
            def gcn_layer(lay, table, dst_shard, S_sb=None, idx_all=None):
                DIN = FIN if lay == 1 else GCN_H
                wnext = {1: w2_sb, 2: w3_sb}.get(lay)
                bias = b_sb[f"b{lay}"]
                nbuf = 2 if lay == 1 else 3
                with tc.tile_pool(name=f"gcn{lay}", bufs=nbuf) as gp, \
                     tc.tile_pool(name=f"gcnsp{lay}", bufs=2) as gsp, \
                     tc.tile_pool(name=f"gcnp{lay}", bufs=2, space="PSUM") as gps, \
                     tc.tile_pool(name=f"gcns{lay}", bufs=2) as sp2:
                    for w in range(NW):
                        g = gp.tile([MAXE, NCT * DIN], bf16, tag="g")
                        if lay == 1:
                            nc.sync.dma_start(g[:], table.ap()[w])
                        else:
                            for j in range(NCT):
                                nc.gpsimd.indirect_dma_start(
                                    out=g[:, j*DIN:(j+1)*DIN], out_offset=None,
                                    in_=table[:],
                                    in_offset=bass.IndirectOffsetOnAxis(
                                        ap=idx_all[:, w*NCT+j: w*NCT+j+1],
                                        axis=0),
                                )
                        ssp_sb = gsp.tile([MAXE, WIN], bf16, tag="ssp")
                        nc.sync.dma_start(ssp_sb[:], SSP[:, w*WIN:(w+1)*WIN])
                        agg = gps.tile([DIN, WIN], f32, tag="agg")
                        nc.tensor.matmul(
                            agg[:], lhsT=g[:, NCELL*DIN:NCT*DIN],
                            rhs=ssp_sb[:], start=True, stop=False)
                        for j in range(NCELL):
                            nc.tensor.matmul(
                                agg[:, j*CELL:(j+1)*CELL],
                                lhsT=g[:, j*DIN:(j+1)*DIN],
                                rhs=S_sb[:, w*WIN + j*CELL: w*WIN + (j+1)*CELL],
                                start=False, stop=True)
                        if lay == 1:
                            aggs = sp2.tile([FIN, WIN], bf16, tag="aggs")
                            nc.vector.tensor_copy(aggs[:], agg[:])
                            h_ps = gps.tile([GCN_H, WIN], f32, tag="hps")
                            nc.tensor.matmul(h_ps[:], lhsT=w1_sb[:], rhs=aggs[:],
                                             start=True, stop=True)
                            hT = sp2.tile([GCN_H, WIN], bf16, tag="hT")
                            nc.scalar.activation(hT[:], h_ps[:], AF.Relu, bias=bias[:])
                        elif lay == 2:
                            hT = sp2.tile([GCN_H, WIN], bf16, tag="hT")
                            nc.scalar.activation(hT[:], agg[:], AF.Relu, bias=bias[:])
                        else:
                            # write h3 directly into token layout:
                            # node v = c*NS + w*512 + loc; u=(v)%5; t=(v%2560)//5;
                            # b_w = w//5; token col = t*BC + b_w; tok row = 32u+f
                            bw = w // 5
                            a0w = (w % 5) * WIN
                            for j in range(5):
                                u = (a0w + j) % 5
                                t0 = (a0w + j) // 5
                                nj = (WIN - j + 4) // 5
                                srcv = agg[:, j:j + (nj - 1) * 5 + 1]
                                srcv.ap = mybir.VecI64Pair([list(srcv.ap[0]), [5, nj]])
                                c0 = u*TB + t0*BC + bw
                                dstv = tokAll[:, c0: c0 + (nj-1)*BC + 1]
                                dstv.ap = mybir.VecI64Pair([list(dstv.ap[0]), [BC, nj]])
                                nc.scalar.activation(dstv, srcv,
                                                     AF.Identity, bias=bias[:])
                        if dst_shard is not None:
                            mn_ps = gps.tile([MAXE, 4 * GCN_H], f32, tag="mnps")
                            for cc in range(4):
                                nc.tensor.matmul(
                                    mn_ps[:, cc*GCN_H:(cc+1)*GCN_H],
                                    lhsT=hT[:, cc*128:(cc+1)*128],
                                    rhs=wnext[:], start=True, stop=True)
                            mn = sp2.tile([MAXE, 4 * GCN_H], bf16, tag="mn")
                            nc.vector.tensor_copy(mn[:], mn_ps[:])
                            nc.sync.dma_start(
                                dst_shard.ap().rearrange(
                                    "(w p c) f -> w p (c f)", w=NW, p=MAXE)[w],
                                mn[:])

            with tc.tile_pool(name="spool", bufs=1) as spool:
                S_sb = spool.tile([MAXE, NW * WIN], bf16)
                nc.sync.dma_start(S_sb[:], ST[:])
                idx_all2 = spool.tile([MAXE, NW * NCT], i32)
                nc.sync.dma_start(idx_all2[:], idxT2[:])
                gcn_layer(1, xg, m2s, S_sb)
                nc.gpsimd.collective_compute(
                    "AllGather", mybir.AluOpType.bypass, replica_groups=RG,
                    ins=[m2s.ap().opt()], outs=[m2f.ap().opt()])
                gcn_layer(2, m2f, m3s, S_sb, idx_all2)
                nc.gpsimd.collective_compute(
                    "AllGather", mybir.AluOpType.bypass, replica_groups=RG,
                    ins=[m3s.ap().opt()], outs=[m3f.ap().opt()])
                gcn_layer(3, m3f, None, S_sb, idx_all2)
                if dbg:
                    nc.sync.dma_start(m2d[:], m2s[:])
                    nc.sync.dma_start(m3d[:], m3s[:])
                    nc.sync.dma_start(gcn3d[:], tokAll[:])

            # ---------------- LSTM ----------------
            with tc.tile_pool(name="lstm", bufs=1) as lp:
                h0f = lp.tile([H, TB], bf16)
                h0b = lp.tile([H, TB], bf16)
                h1f = lp.tile([H, TB], bf16)
                h1b = lp.tile([H, TB], bf16)
                ones1 = lp.tile([1, WIN], bf16)
                nc.vector.memset(ones1[:], 1.0)
                zeroBC = lp.tile([H, BC], bf16)
                nc.vector.memset(zeroBC[:], 0.0)
                ident_sb = lp.tile([128, 128], bf16)
                nc.sync.dma_start(ident_sb[:], identT[:])

                wih_sb = {}
                for key in ("0f", "0b"):
                    wchunks = lp.tile([GCN_H, 5 * 4 * H], bf16, tag=f"wih5{key}",
                                      name=f"wih5{key}")
                    for u in range(5):
                        nc.sync.dma_start(
                            wchunks[:, u*4*H:(u+1)*4*H], wih[key][u*32:(u+1)*32])
                    wihBias = lp.tile([1, 4 * H], bf16, tag=f"wihC{key}",
                                      name=f"wihC{key}")
                    nc.sync.dma_start(wihBias[:], wih[key][LSTM_IN:LSTM_IN+1])
                    wih_sb[key] = (wchunks, wihBias)
                for key in ("1f", "1b"):
                    wihA = lp.tile([128, 4 * H], bf16, tag=f"wihA{key}", name=f"wihA{key}")
                    nc.sync.dma_start(wihA[:], wih[key][0:128])
                    wihB = lp.tile([128, 4 * H], bf16, tag=f"wihB{key}", name=f"wihB{key}")
                    nc.sync.dma_start(wihB[:], wih[key][128:256])
                    wihBias = lp.tile([1, 4 * H], bf16, tag=f"wihC{key}", name=f"wihC{key}")
                    nc.sync.dma_start(wihBias[:], wih[key][256:257])
                    wih_sb[key] = (wihA, wihB, wihBias)
                whh_sb = {}
                for key in ("0f", "0b", "1f", "1b"):
                    whhT = lp.tile([H, 4 * H], bf16, tag=f"whh{key}")
                    nc.sync.dma_start(whhT[:], whh[key][:])
                    whh_sb[key] = whhT

                def xw_precompute(ph, chunks_f, chunks_b):
                    # chunks: per dir list of (wih_rows_slice, rhs_ap) K-chunks
                    with tc.tile_pool(name=f"xw{ph}", bufs=3) as xp, \
                         tc.tile_pool(name=f"xwp{ph}", bufs=2, space="PSUM") as xps:
                        for dr, chunks in (("f", chunks_f), ("b", chunks_b)):
                            for G in range(4):
                                blk = GBLK[G]
                                for ct in range(TB // WIN):
                                    ps = xps.tile([H, WIN], f32, tag="ps")
                                    nchunks = len(chunks)
                                    for ci, (lhsT, rhs) in enumerate(chunks):
                                        rr = (rhs[:, 0:WIN] if rhs.shape[0] == 1
                                              else rhs[:, ct*WIN:(ct+1)*WIN])
                                        nc.tensor.matmul(
                                            ps[:], lhsT=lhsT[:, G*H:(G+1)*H],
                                            rhs=rr,
                                            start=(ci == 0), stop=(ci == nchunks-1))
                                    sb = xp.tile([H, WIN], bf16, tag="sb")
                                    nc.vector.tensor_copy(sb[:], ps[:])
                                    # layout (g, t, b): gate block contiguous
                                    nc.sync.dma_start(
                                        xwd[ph+dr].ap()[:, blk*TB + ct*WIN: blk*TB + (ct+1)*WIN],
                                        sb[:])

                def lstm_phase(ph, hf_st, hb_st):
                    # fused f/b: gate-major PSUM blocks [i_f i_b f_f f_b o_f
                    # o_b g_f g_b] (BC cols each); sigmoid(x)=0.5+0.5*tanh(x)
                    # with the 0.5 pre-folded into i,f,o weights at host.
                    with tc.tile_pool(name=f"lph{ph}", bufs=3) as php, \
                         tc.tile_pool(name=f"lxw{ph}", bufs=2) as pxw, \
                         tc.tile_pool(name=f"lpc{ph}", bufs=1) as phc, \
                         tc.tile_pool(name=f"lps{ph}", bufs=4, space="PSUM") as phps:
                        cst = phc.tile([H, 2 * BC], f32, tag="cst")
                        nc.vector.memset(cst[:], 0.0)
                        halfs = phc.tile([H, 6 * BC], f32, tag="halfs")
                        nc.vector.memset(halfs[:], 0.5)
                        xw_b = {}
                        for t in range(T):
                            tt = T - 1 - t
                            if t % XWCH == 0:
                                for dr in ("f", "b"):
                                    xw_b[dr] = pxw.tile([H, 4, XWCH * BC], bf16, tag=f"xw{dr}", name=f"xwb{dr}")
                                    base = t if dr == "f" else tt - XWCH + 1
                                    for Gb in range(4):
                                        nc.sync.dma_start(
                                            xw_b[dr][:, Gb, :],
                                            xwd[ph+dr].ap()[:, Gb*TB + base*BC: Gb*TB + (base+XWCH)*BC])
                            of = (t % XWCH) * BC
                            ob = (XWCH - 1 - (t % XWCH)) * BC
                            gp_ = phps.tile([H, 8, BC], f32, tag="gps", name="gps")
                            for d_i, dr in ((0, "f"), (1, "b")):
                                st = hf_st if d_i == 0 else hb_st
                                tm = t if d_i == 0 else tt
                                if t == 0:
                                    hprev = zeroBC[:]
                                elif d_i == 0:
                                    hprev = st[:, (tm-1)*BC:tm*BC]
                                else:
                                    hprev = st[:, (tm+1)*BC:(tm+2)*BC]
                                wt = whh_sb[ph + dr]
                                for G in range(4):
                                    nc.tensor.matmul(
                                        gp_[:, 2*GBLK[G] + d_i, :],
                                        lhsT=wt[:, G*H:(G+1)*H], rhs=hprev,
                                        start=True, stop=True)
                            gf_ = php.tile([H, 8, BC], f32, tag="gf", name="gf")
                            nc.vector.tensor_add(
                                gf_[:, 0:8:2, :], gp_[:, 0:8:2, :],
                                xw_b["f"][:, :, of:of+BC])
                            nc.vector.tensor_add(
                                gf_[:, 1:8:2, :], gp_[:, 1:8:2, :],
                                xw_b["b"][:, :, ob:ob+BC])
                            sa = php.tile([H, 8 * BC], f32, tag="sa", name="sa")
                            nc.scalar.activation(
                                sa[:], gf_[:].rearrange("h g b -> h (g b)"),
                                AF.Tanh)
                            # sigmoid reconstruction for the 6 i/f/o blocks
                            nc.vector.scalar_tensor_tensor(
                                out=sa[:, 0:6*BC], in0=sa[:, 0:6*BC],
                                scalar=0.5, in1=halfs[:],
                                op0=mybir.AluOpType.mult,
                                op1=mybir.AluOpType.add)
                            mm = php.tile([H, 2 * BC], f32, tag="mm", name="mm")
                            nc.vector.tensor_mul(mm[:], sa[:, 0:2*BC], sa[:, 6*BC:8*BC])
                            nc.vector.tensor_mul(cst[:], cst[:], sa[:, 2*BC:4*BC])
                            nc.vector.tensor_add(cst[:], cst[:], mm[:])
                            tc_ = php.tile([H, 2 * BC], f32, tag="tc", name="tc")
                            nc.scalar.activation(tc_[:], cst[:], AF.Tanh)
                            nc.vector.tensor_mul(
                                hf_st[:, t*BC:(t+1)*BC], sa[:, 4*BC:5*BC], tc_[:, 0:BC])
                            nc.vector.tensor_mul(
                                hb_st[:, tt*BC:(tt+1)*BC], sa[:, 5*BC:6*BC], tc_[:, BC:2*BC])

                def chunks0(key):
                    wchunks, cbias = wih_sb[key]
                    ch = [(wchunks[:, u*4*H:(u+1)*4*H], tokAll[:, u*TB:(u+1)*TB])
                          for u in range(5)]
                    ch.append((cbias, ones1))
                    return ch

                def chunks_for(key, rA, rB):
                    a, bwt, cbias = wih_sb[key]
                    return [(a, rA), (bwt, rB), (cbias, ones1)]
                xw_precompute("0", chunks0("0f"), chunks0("0b"))
                lstm_phase("0", h0f, h0b)
                if dbg:
                    nc.sync.dma_start(h0d.ap()[:, 0:TB], h0f[:])
                    nc.sync.dma_start(h0d.ap()[:, TB:2*TB], h0b[:])
                xw_precompute("1", chunks_for("1f", h0f, h0b),
                              chunks_for("1b", h0f, h0b))
                lstm_phase("1", h1f, h1b)

                wfc_a = lp.tile([H, NCLS], bf16)
                nc.sync.dma_start(wfc_a[:], wfc[0:H])
                wfc_b = lp.tile([H, NCLS], bf16)
                nc.sync.dma_start(wfc_b[:], wfc[H:])
                bfc_sb = lp.tile([BC, NCLS], f32)
                nc.sync.dma_start(bfc_sb[:], bfc[:])
                with tc.tile_pool(name="fcps", bufs=1, space="PSUM") as fps:
                    fc_ps = fps.tile([BC, NCLS], f32)
                    nc.tensor.matmul(fc_ps[:], lhsT=h1f[:, (T-1)*BC:T*BC],
                                     rhs=wfc_a[:], start=True, stop=False)
                    nc.tensor.matmul(fc_ps[:], lhsT=h1b[:, (T-1)*BC:T*BC],
                                     rhs=wfc_b[:], start=False, stop=True)
                    fc_sb = lp.tile([BC, NCLS], f32)
                    nc.vector.tensor_add(fc_sb[:], fc_ps[:], bfc_sb[:])
                    nc.sync.dma_start(out[:], fc_sb[:])
    return nc


# =====================================================================
# Entry
# =====================================================================

_CACHED = {}

def kernel(**inputs):
    xg_cores, idx_cores, S_cores = _prep_graph(
        inputs["x"], inputs["edge_src"], inputs["edge_dst"])
    wts = _prep_weights(inputs)

    if "nc" not in _CACHED:
        nc_new = build_kernel()
        if not nc_new.is_finalized():
            nc_new.finalize()
        _CACHED["nc"] = nc_new
    nc = _CACHED["nc"]

    in_maps = []
    for c in range(NCORES):
        m = dict(
            xg=xg_cores[c], idxT2=idx_cores[c],
            ST=S_cores[c][0], SSP=S_cores[c][1],
            identT=np.eye(128, dtype=np.float32).astype(BF16),
            W1=wts["W1"], W2=wts["W2"], W3=wts["W3"],
            b1=wts["b1"], b2=wts["b2"], b3=wts["b3"],
            wfc=wts["wfc"], bfc=wts["bfc"],
        )
        for tag in ("0", "1"):
            for dr in ("f", "b"):
                m[f"wih{tag}{dr}"] = wts[f"wih{tag}{dr}"]
                m[f"whh{tag}{dr}"] = wts[f"whh{tag}{dr}"]
        in_maps.append(m)

    res = run_bass_kernel_spmd(nc, in_maps, core_ids=list(range(NCORES)),
                               trace=os.environ.get("KTRACE", "0") == "1")
    kernel.last_result = res
    kernel.last_debug = res.results if os.environ.get("KDEBUG") == "1" else None
    outs = [res.results[c]["out"] for c in range(NCORES)]
    return np.concatenate(outs, axis=0).astype(np.float32)



# revision 14
# speedup vs baseline: 1.0436x; 1.0436x over previous
"""Trainium2 Bass kernel for GCN(3-layer) + BiLSTM(2-layer) + FC.

Self-contained: hardcodes all shapes; needs /opt/trn_rl_repo (concourse) only.

Architecture (8 NeuronCores, SPMD):
 - Nodes sharded by dst range (40960/core, graph-aligned).  A_hat = D^-1/2 (A+I) D^-1/2.
 - GCN layer 1: src features pre-gathered on host per (window, cell, slot) ->
   contiguous HWDGE loads, no indirect DMA.
 - GCN layers 2/3: ONE batched indirect gather per 512-node window (idx block
   [128, 43]), scatter-add via PE matmuls against host-built sparse cell
   blocks S (norm weights folded in), fused bias/relu on ACT, next-layer
   premultiply m_{k+1} = h_k @ W on PE, AllGather of m shards between layers.
 - BiLSTM: batch-sharded 16 seq/core, feature-major state (H=128 partitions),
   per-step gate matmuls on PE, sigmoid/tanh on ACT, cell math on DVE.
 - All contiguous DMA on SP HWDGE; GpSimd (SWDGE) reserved for the two
   indirect gathers per layer-window and collectives.
"""
import sys, os
sys.path.insert(0, "/opt/trn_rl_repo")
import numpy as np
import ml_dtypes

import concourse.bass as bass
import concourse.bacc as bacc
import concourse.tile as tile
from concourse import mybir
from concourse.bass_utils import run_bass_kernel_spmd

BF16 = ml_dtypes.bfloat16
AF = mybir.ActivationFunctionType

# ---- problem constants (hardcoded) ----
B, T, FEAT, H, GCN_H, NCLS = 128, 512, 320, 128, 32, 10
N = B * 5 * T            # 327680
FIN = FEAT // 5          # 64
LSTM_IN = 5 * GCN_H      # 160
NCORES = 8
NS = N // NCORES         # 40960 nodes/core
WIN = 512                # window cols
NW = NS // WIN           # 80 windows/core
CELL = 16
NCELL = 32               # regular cells per window (16 cols each)
NCT = NCELL + 1          # + 1 spill cell (full-window S, accumulated last)
MAXE = 128               # max edges per cell slot-block
BC = B // NCORES         # 16 sequences per core
TB = T * BC              # 8192 token-cols per core
XWCH = 16                # lstm xw prefetch chunk (steps)


# =====================================================================
# Host preprocessing
# =====================================================================

def _prep_graph(x, edge_src, edge_dst):
    src = np.asarray(edge_src, np.int64)
    dst = np.asarray(edge_dst, np.int64)
    deg = np.bincount(dst, minlength=N).astype(np.float64) + 1.0
    dinv = 1.0 / np.sqrt(deg)
    sl = np.arange(N, dtype=np.int64)
    s_all = np.concatenate([src, sl])
    d_all = np.concatenate([dst, sl])
    w_all = (dinv[s_all] * dinv[d_all]).astype(np.float32)
    x_bf = np.asarray(x, np.float32).astype(BF16)

    # layers 2/3 gather-index remap: m rows are stored (window, p, c) packed
    vv = np.arange(N, dtype=np.int64)
    loc = vv % WIN
    M_REMAP = (vv // WIN) * WIN + (loc % 128) * 4 + loc // 128
    xg_cores, idx_cores, S_cores = [], [], []
    for c in range(NCORES):
        m = (d_all // NS) == c
        s_c = s_all[m]; d_c = d_all[m] - c * NS; w_c = w_all[m]
        wid = d_c // WIN
        cj = (d_c % WIN) // CELL
        cell_id = wid * NCELL + cj
        order = np.argsort(cell_id, kind="stable")
        s_c = s_c[order]; d_c = d_c[order]; w_c = w_c[order]
        cell_id = cell_id[order]
        counts = np.bincount(cell_id, minlength=NW * NCELL)
        starts = np.concatenate([[0], np.cumsum(counts)])
        rank = np.arange(len(s_c)) - starts[cell_id]
        wi = cell_id // NCELL
        ji = cell_id % NCELL
        reg = rank < MAXE
        # spill: per-window running slot for overflow edges
        sp = ~reg
        spill_slot = np.zeros(len(s_c), np.int64)
        ww = wi[sp]
        srt = np.argsort(ww, kind="stable")
        sslot = np.zeros(sp.sum(), np.int64)
        wcounts = np.bincount(ww, minlength=NW)
        wstarts = np.concatenate([[0], np.cumsum(wcounts)])
        sslot[srt] = np.arange(sp.sum()) - wstarts[ww[srt]]
        assert sp.sum() == 0 or sslot.max() < MAXE
        idx_arr = np.zeros((NW, NCT, MAXE), np.int32)
        idx_arr[wi[reg], ji[reg], rank[reg]] = s_c[reg]
        idx_arr[wi[sp], NCELL, sslot] = s_c[sp]
        # regular S: (MAXE, NW*512): cell j owns cols [w*512+j*16, +16)
        S_dev = np.zeros((MAXE, NW * WIN), np.float32)
        S_dev[rank[reg], wi[reg] * WIN + (d_c[reg] % WIN)] = w_c[reg]
        # spill S: (MAXE, NW*512) full-window routing for spill slots
        Ssp = np.zeros((MAXE, NW * WIN), np.float32)
        Ssp[sslot, wi[sp] * WIN + (d_c[sp] % WIN)] = w_c[sp]
        # layer-1 host pre-gather in (w, slot, cell) blocks
        xg = np.ascontiguousarray(
            x_bf[idx_arr].transpose(0, 2, 1, 3).reshape(NW, MAXE, NCT * FIN))
        xg_cores.append(xg)
        i1 = np.ascontiguousarray(
            idx_arr.transpose(2, 0, 1).reshape(MAXE, NW * NCT))
        i23 = M_REMAP[i1].astype(np.int32)
        idx_cores.append(i23)
        S_cores.append((S_dev.astype(BF16), Ssp.astype(BF16)))
    return xg_cores, idx_cores, S_cores


def _prep_weights(inp):
    d = {}
    d["W1"] = np.asarray(inp["W1"], np.float32).astype(BF16)
    d["W2"] = np.asarray(inp["W2"], np.float32).astype(BF16)
    d["W3"] = np.asarray(inp["W3"], np.float32).astype(BF16)
    for k in ("b1", "b2", "b3"):
        d[k] = np.asarray(inp[k], np.float32).reshape(GCN_H, 1)

    # gates use native ACT Sigmoid/Tanh: no weight pre-scaling
    GSCL = np.repeat([1.0, 1.0, 1.0, 1.0], H).astype(np.float32)

    def pack_ih(Wih, bih, bhh):
        Wih = np.asarray(Wih, np.float32)
        inn = Wih.shape[1]
        o = np.zeros((inn + 1, 4 * H), np.float32)
        o[:inn] = Wih.T
        o[inn] = np.asarray(bih, np.float32) + np.asarray(bhh, np.float32)
        return (o * GSCL).astype(BF16)

    def pack_hh(Whh):
        Whh = np.asarray(Whh, np.float32)
        return (np.concatenate(
            [Whh[g*H:(g+1)*H, :].T for g in range(4)], axis=1) * GSCL).astype(BF16)

    for tag in ("0", "1"):
        for dr in ("f", "b"):
            d[f"wih{tag}{dr}"] = pack_ih(
                inp[f"Wih{tag}{dr}"], inp[f"bih{tag}{dr}"], inp[f"bhh{tag}{dr}"])
            d[f"whh{tag}{dr}"] = pack_hh(inp[f"Whh{tag}{dr}"])
    d["wfc"] = np.asarray(inp["Wfc"], np.float32).astype(BF16)
    d["bfc"] = np.broadcast_to(
        np.asarray(inp["bfc"], np.float32), (BC, NCLS)).copy()
    return d


# =====================================================================
# Bass kernel builder
# =====================================================================

# torch gate order i,f,g,o -> device col blocks [i f o g]
GBLK = {0: 0, 1: 1, 2: 3, 3: 2}
USE_IDMM = os.environ.get("KIDMM", "0") == "1"
USE_GPS = os.environ.get("KGPS", "0") == "1"


def build_kernel():
    nc = bacc.Bacc(None, num_devices=NCORES)
    dt = mybir.dt
    f32, bf16, i32 = dt.float32, dt.bfloat16, dt.int32

    xg = nc.dram_tensor("xg", [NW, MAXE, NCT * FIN], bf16, kind="ExternalInput")
    idxT2 = nc.dram_tensor("idxT2", [MAXE, NW * NCT], i32, kind="ExternalInput")
    identT = nc.dram_tensor("identT", [128, 128], bf16, kind="ExternalInput")
    ST = nc.dram_tensor("ST", [MAXE, NW * WIN], bf16, kind="ExternalInput")
    SSP = nc.dram_tensor("SSP", [MAXE, NW * WIN], bf16, kind="ExternalInput")
    W1 = nc.dram_tensor("W1", [FIN, GCN_H], bf16, kind="ExternalInput")
    W2 = nc.dram_tensor("W2", [GCN_H, GCN_H], bf16, kind="ExternalInput")
    W3 = nc.dram_tensor("W3", [GCN_H, GCN_H], bf16, kind="ExternalInput")
    b1 = nc.dram_tensor("b1", [GCN_H, 1], f32, kind="ExternalInput")
    b2 = nc.dram_tensor("b2", [GCN_H, 1], f32, kind="ExternalInput")
    b3 = nc.dram_tensor("b3", [GCN_H, 1], f32, kind="ExternalInput")
    wih, whh = {}, {}
    for tag, inn in (("0", LSTM_IN), ("1", 2 * H)):
        for dr in ("f", "b"):
            wih[tag+dr] = nc.dram_tensor(f"wih{tag}{dr}", [inn + 1, 4 * H], bf16, kind="ExternalInput")
            whh[tag+dr] = nc.dram_tensor(f"whh{tag}{dr}", [H, 4 * H], bf16, kind="ExternalInput")
    wfc = nc.dram_tensor("wfc", [2 * H, NCLS], bf16, kind="ExternalInput")
    bfc = nc.dram_tensor("bfc", [BC, NCLS], f32, kind="ExternalInput")
    out = nc.dram_tensor("out", [BC, NCLS], f32, kind="ExternalOutput")

    dbg = os.environ.get("KDEBUG", "0") == "1"
    m2s = nc.dram_tensor("m2s", [NS, GCN_H], bf16)
    m3s = nc.dram_tensor("m3s", [NS, GCN_H], bf16)
    if dbg:
        m2d = nc.dram_tensor("m2d", [NS, GCN_H], bf16, kind="ExternalOutput")
        m3d = nc.dram_tensor("m3d", [NS, GCN_H], bf16, kind="ExternalOutput")
        gcn3d = nc.dram_tensor("gcn3d", [GCN_H, 5 * TB], bf16, kind="ExternalOutput")
        h0d = nc.dram_tensor("h0d", [H, 2 * TB], bf16, kind="ExternalOutput")
    m2f = nc.dram_tensor("m2f", [N, GCN_H], bf16, addr_space="Shared")
    m3f = nc.dram_tensor("m3f", [N, GCN_H], bf16, addr_space="Shared")
    # xw tensors: per phase, per dir: (H, T*4*BC): col = t*64 + blk*16 + b
    xwd = {ph+dr: nc.dram_tensor(f"xw{ph}{dr}", [H, T * 4 * BC], bf16)
           for ph in ("0", "1") for dr in ("f", "b")}

    RG = [list(range(NCORES))]

    with tile.TileContext(nc) as tc:
        with tc.tile_pool(name="persist", bufs=1) as pp:
            w1_sb = pp.tile([FIN, GCN_H], bf16)
            nc.sync.dma_start(w1_sb[:], W1[:])
            w2_sb = pp.tile([GCN_H, GCN_H], bf16)
            nc.sync.dma_start(w2_sb[:], W2[:])
            w3_sb = pp.tile([GCN_H, GCN_H], bf16)
            nc.sync.dma_start(w3_sb[:], W3[:])
            b_sb = {}
            for nm, t_ in (("b1", b1), ("b2", b2), ("b3", b3)):
                bias_tile = pp.tile([GCN_H, 1], f32, tag=nm)
                b_sb[nm] = bias_tile
                nc.sync.dma_start(b_sb[nm][:], t_[:])
            tokAll = pp.tile([GCN_H, 5 * TB], bf16)

            # ---------------- GCN ----------------
            def gcn_layer(lay, table, dst_shard, S_sb=None, idx_all=None):
                DIN = FIN if lay == 1 else GCN_H
                wnext = {1: w2_sb, 2: w3_sb}.get(lay)
                bias = b_sb[f"b{lay}"]
                nbuf = 2 if lay == 1 else 3
                with tc.tile_pool(name=f"gcn{lay}", bufs=nbuf) as gp, \
                     tc.tile_pool(name=f"gcnsp{lay}", bufs=2) as gsp, \
                     tc.tile_pool(name=f"gcnp{lay}", bufs=2, space="PSUM") as gps, \
                     tc.tile_pool(name=f"gcns{lay}", bufs=2) as sp2:
                    for w in range(NW):
                        g = gp.tile([MAXE, NCT * DIN], bf16, tag="g")
                        if lay == 1:
                            nc.sync.dma_start(g[:], table.ap()[w])
                        else:
                            for j in range(NCT):
                                nc.gpsimd.indirect_dma_start(
                                    out=g[:, j*DIN:(j+1)*DIN], out_offset=None,
                                    in_=table[:],
                                    in_offset=bass.IndirectOffsetOnAxis(
                                        ap=idx_all[:, w*NCT+j: w*NCT+j+1],
                                        axis=0),
                                )
                        ssp_sb = gsp.tile([MAXE, WIN], bf16, tag="ssp")
                        nc.sync.dma_start(ssp_sb[:], SSP[:, w*WIN:(w+1)*WIN])
                        agg = gps.tile([DIN, WIN], f32, tag="agg")
                        nc.tensor.matmul(
                            agg[:], lhsT=g[:, NCELL*DIN:NCT*DIN],
                            rhs=ssp_sb[:], start=True, stop=False)
                        for j in range(NCELL):
                            nc.tensor.matmul(
                                agg[:, j*CELL:(j+1)*CELL],
                                lhsT=g[:, j*DIN:(j+1)*DIN],
                                rhs=S_sb[:, w*WIN + j*CELL: w*WIN + (j+1)*CELL],
                                start=False, stop=True)
                        if lay == 1:
                            aggs = sp2.tile([FIN, WIN], bf16, tag="aggs")
                            nc.vector.tensor_copy(aggs[:], agg[:])
                            h_ps = gps.tile([GCN_H, WIN], f32, tag="hps")
                            nc.tensor.matmul(h_ps[:], lhsT=w1_sb[:], rhs=aggs[:],
                                             start=True, stop=True)
                            hT = sp2.tile([GCN_H, WIN], bf16, tag="hT")
                            nc.scalar.activation(hT[:], h_ps[:], AF.Relu, bias=bias[:])
                        elif lay == 2:
                            hT = sp2.tile([GCN_H, WIN], bf16, tag="hT")
                            nc.scalar.activation(hT[:], agg[:], AF.Relu, bias=bias[:])
                        else:
                            # write h3 directly into token layout:
                            # node v = c*NS + w*512 + loc; u=(v)%5; t=(v%2560)//5;
                            # b_w = w//5; token col = t*BC + b_w; tok row = 32u+f
                            bw = w // 5
                            a0w = (w % 5) * WIN
                            for j in range(5):
                                u = (a0w + j) % 5
                                t0 = (a0w + j) // 5
                                nj = (WIN - j + 4) // 5
                                srcv = agg[:, j:j + (nj - 1) * 5 + 1]
                                srcv.ap = mybir.VecI64Pair([list(srcv.ap[0]), [5, nj]])
                                c0 = u*TB + t0*BC + bw
                                dstv = tokAll[:, c0: c0 + (nj-1)*BC + 1]
                                dstv.ap = mybir.VecI64Pair([list(dstv.ap[0]), [BC, nj]])
                                nc.scalar.activation(dstv, srcv,
                                                     AF.Identity, bias=bias[:])
                        if dst_shard is not None:
                            mn_ps = gps.tile([MAXE, 4 * GCN_H], f32, tag="mnps")
                            for cc in range(4):
                                nc.tensor.matmul(
                                    mn_ps[:, cc*GCN_H:(cc+1)*GCN_H],
                                    lhsT=hT[:, cc*128:(cc+1)*128],
                                    rhs=wnext[:], start=True, stop=True)
                            mn = sp2.tile([MAXE, 4 * GCN_H], bf16, tag="mn")
                            nc.vector.tensor_copy(mn[:], mn_ps[:])
                            nc.sync.dma_start(
                                dst_shard.ap().rearrange(
                                    "(w p c) f -> w p (c f)", w=NW, p=MAXE)[w],
                                mn[:])

            with tc.tile_pool(name="spool", bufs=1) as spool:
                S_sb = spool.tile([MAXE, NW * WIN], bf16)
                nc.sync.dma_start(S_sb[:], ST[:])
                idx_all2 = spool.tile([MAXE, NW * NCT], i32)
                nc.sync.dma_start(idx_all2[:], idxT2[:])
                gcn_layer(1, xg, m2s, S_sb)
                nc.gpsimd.collective_compute(
                    "AllGather", mybir.AluOpType.bypass, replica_groups=RG,
                    ins=[m2s.ap().opt()], outs=[m2f.ap().opt()])
                gcn_layer(2, m2f, m3s, S_sb, idx_all2)
                nc.gpsimd.collective_compute(
                    "AllGather", mybir.AluOpType.bypass, replica_groups=RG,
                    ins=[m3s.ap().opt()], outs=[m3f.ap().opt()])
                gcn_layer(3, m3f, None, S_sb, idx_all2)
                if dbg:
                    nc.sync.dma_start(m2d[:], m2s[:])
                    nc.sync.dma_start(m3d[:], m3s[:])
                    nc.sync.dma_start(gcn3d[:], tokAll[:])

            # ---------------- LSTM ----------------
            with tc.tile_pool(name="lstm", bufs=1) as lp:
                h0f = lp.tile([H, TB], bf16)
                h0b = lp.tile([H, TB], bf16)
                h1f = lp.tile([H, TB], bf16)
                h1b = lp.tile([H, TB], bf16)
                ones1 = lp.tile([1, WIN], bf16)
                nc.vector.memset(ones1[:], 1.0)
                zeroBC = lp.tile([H, BC], bf16)
                nc.vector.memset(zeroBC[:], 0.0)
                ident_sb = lp.tile([128, 128], bf16)
                nc.sync.dma_start(ident_sb[:], identT[:])

                wih_sb = {}
                for key in ("0f", "0b"):
                    wchunks = lp.tile([GCN_H, 5 * 4 * H], bf16, tag=f"wih5{key}",
                                      name=f"wih5{key}")
                    for u in range(5):
                        nc.sync.dma_start(
                            wchunks[:, u*4*H:(u+1)*4*H], wih[key][u*32:(u+1)*32])
                    wihBias = lp.tile([1, 4 * H], bf16, tag=f"wihC{key}",
                                      name=f"wihC{key}")
                    nc.sync.dma_start(wihBias[:], wih[key][LSTM_IN:LSTM_IN+1])
                    wih_sb[key] = (wchunks, wihBias)
                for key in ("1f", "1b"):
                    wihA = lp.tile([128, 4 * H], bf16, tag=f"wihA{key}", name=f"wihA{key}")
                    nc.sync.dma_start(wihA[:], wih[key][0:128])
                    wihB = lp.tile([128, 4 * H], bf16, tag=f"wihB{key}", name=f"wihB{key}")
                    nc.sync.dma_start(wihB[:], wih[key][128:256])
                    wihBias = lp.tile([1, 4 * H], bf16, tag=f"wihC{key}", name=f"wihC{key}")
                    nc.sync.dma_start(wihBias[:], wih[key][256:257])
                    wih_sb[key] = (wihA, wihB, wihBias)
                whh_sb = {}
                for key in ("0f", "0b", "1f", "1b"):
                    whhT = lp.tile([H, 4 * H], bf16, tag=f"whh{key}")
                    nc.sync.dma_start(whhT[:], whh[key][:])
                    whh_sb[key] = whhT

                def xw_precompute(ph, chunks_f, chunks_b):
                    # chunks: per dir list of (wih_rows_slice, rhs_ap) K-chunks
                    with tc.tile_pool(name=f"xw{ph}", bufs=3) as xp, \
                         tc.tile_pool(name=f"xwp{ph}", bufs=2, space="PSUM") as xps:
                        for dr, chunks in (("f", chunks_f), ("b", chunks_b)):
                            for G in range(4):
                                blk = GBLK[G]
                                for ct in range(TB // WIN):
                                    ps = xps.tile([H, WIN], f32, tag="ps")
                                    nchunks = len(chunks)
                                    for ci, (lhsT, rhs) in enumerate(chunks):
                                        rr = (rhs[:, 0:WIN] if rhs.shape[0] == 1
                                              else rhs[:, ct*WIN:(ct+1)*WIN])
                                        nc.tensor.matmul(
                                            ps[:], lhsT=lhsT[:, G*H:(G+1)*H],
                                            rhs=rr,
                                            start=(ci == 0), stop=(ci == nchunks-1))
                                    sb = xp.tile([H, WIN], bf16, tag="sb")
                                    nc.vector.tensor_copy(sb[:], ps[:])
                                    # layout (g, t, b): gate block contiguous
                                    nc.sync.dma_start(
                                        xwd[ph+dr].ap()[:, blk*TB + ct*WIN: blk*TB + (ct+1)*WIN],
                                        sb[:])

                def lstm_phase(ph, hf_st, hb_st):
                    # split f/b chains; gates scaled so sigmoid(x)=0.5+0.5*tanh(x)
                    # (0.5 factor folded into i,f,o weights at host). Gate block
                    # order in PSUM: [i f o g].
                    with tc.tile_pool(name=f"lph{ph}", bufs=3) as php, \
                         tc.tile_pool(name=f"lxw{ph}", bufs=2) as pxw, \
                         tc.tile_pool(name=f"lpc{ph}", bufs=1) as phc, \
                         tc.tile_pool(name=f"lps{ph}", bufs=4, space="PSUM") as phps:
                        cst = {}
                        for dr in ("f", "b"):
                            cst[dr] = phc.tile([H, BC], f32, tag=f"c{dr}", name=f"cst{dr}")
                            nc.vector.memset(cst[dr][:], 0.0)
                        halfs = phc.tile([H, 3 * BC], f32, tag="halfs")
                        nc.vector.memset(halfs[:], 0.5)
                        xw_b = {}
                        for t in range(T):
                            tt = T - 1 - t
                            if t % XWCH == 0:
                                for dr in ("f", "b"):
                                    xw_b[dr] = pxw.tile([H, 4, XWCH * BC], bf16, tag=f"xw{dr}", name=f"xwb{dr}")
                                    base = t if dr == "f" else tt - XWCH + 1
                                    for Gb in range(4):
                                        nc.sync.dma_start(
                                            xw_b[dr][:, Gb, :],
                                            xwd[ph+dr].ap()[:, Gb*TB + base*BC: Gb*TB + (base+XWCH)*BC])
                            of = (t % XWCH) * BC
                            ob = (XWCH - 1 - (t % XWCH)) * BC
                            gate_ps, sact = {}, {}
                            for d_i, dr in ((0, "f"), (1, "b")):
                                st = hf_st if d_i == 0 else hb_st
                                tm = t if d_i == 0 else tt
                                if t == 0:
                                    hprev = zeroBC[:]
                                elif d_i == 0:
                                    hprev = st[:, (tm-1)*BC:tm*BC]
                                else:
                                    hprev = st[:, (tm+1)*BC:(tm+2)*BC]
                                wt = whh_sb[ph + dr]
                                gp_ = phps.tile([H, 4 * BC], f32, tag=f"g{dr}", name=f"gps{dr}")
                                gate_ps[dr] = gp_
                                off = of if d_i == 0 else ob
                                if USE_IDMM:
                                    for G in range(4):
                                        blk = GBLK[G]
                                        nc.tensor.matmul(
                                            gp_[:, blk*BC:(blk+1)*BC],
                                            lhsT=wt[:, G*H:(G+1)*H], rhs=hprev,
                                            start=True, stop=False)
                                    nc.tensor.matmul(
                                        gp_[:], lhsT=ident_sb[:],
                                        rhs=xw_b[dr][:, :, off:off+BC],
                                        start=False, stop=True)
                                else:
                                    for G in range(4):
                                        blk = GBLK[G]
                                        nc.tensor.matmul(
                                            gp_[:, blk*BC:(blk+1)*BC],
                                            lhsT=wt[:, G*H:(G+1)*H], rhs=hprev,
                                            start=True, stop=True)
                            for dr in ("f", "b"):
                                sa = php.tile([H, 4 * BC], f32, tag=f"sa{dr}", name=f"sa{dr}")
                                sact[dr] = sa
                                off = of if dr == "f" else ob
                                gf_ = php.tile([H, 4 * BC], f32, tag=f"gf{dr}", name=f"gf{dr}")
                                nc.vector.tensor_add(
                                    gf_[:].rearrange("h (g b) -> h g b", g=4),
                                    gate_ps[dr][:].rearrange("h (g b) -> h g b", g=4),
                                    xw_b[dr][:, :, off:off+BC])
                                nc.scalar.activation(sa[:, 0:3*BC], gf_[:, 0:3*BC],
                                                     AF.Sigmoid)
                                nc.scalar.activation(sa[:, 3*BC:4*BC],
                                                     gf_[:, 3*BC:4*BC], AF.Tanh)
                            mm = {}
                            for dr in ("f", "b"):
                                sa = sact[dr]
                                mm[dr] = php.tile([H, BC], f32, tag=f"mm{dr}", name=f"mmt{dr}")
                                nc.vector.tensor_mul(mm[dr][:], sa[:, 0:BC], sa[:, 3*BC:4*BC])
                            eng = nc.gpsimd if USE_GPS else nc.vector
                            for dr in ("f", "b"):
                                eng.tensor_mul(cst[dr][:], cst[dr][:], sact[dr][:, BC:2*BC])
                            for dr in ("f", "b"):
                                eng.tensor_add(cst[dr][:], cst[dr][:], mm[dr][:])
                            tc_ = {}
                            for dr in ("f", "b"):
                                tc_[dr] = php.tile([H, BC], f32, tag=f"tc{dr}", name=f"tct{dr}")
                                nc.scalar.activation(tc_[dr][:], cst[dr][:], AF.Tanh)
                            nc.vector.tensor_mul(
                                hf_st[:, t*BC:(t+1)*BC], sact["f"][:, 2*BC:3*BC], tc_["f"][:])
                            nc.vector.tensor_mul(
                                hb_st[:, tt*BC:(tt+1)*BC], sact["b"][:, 2*BC:3*BC], tc_["b"][:])

                def chunks0(key):
                    wchunks, cbias = wih_sb[key]
                    ch = [(wchunks[:, u*4*H:(u+1)*4*H], tokAll[:, u*TB:(u+1)*TB])
                          for u in range(5)]
                    ch.append((cbias, ones1))
                    return ch

                def chunks_for(key, rA, rB):
                    a, bwt, cbias = wih_sb[key]
                    return [(a, rA), (bwt, rB), (cbias, ones1)]
                xw_precompute("0", chunks0("0f"), chunks0("0b"))
                lstm_phase("0", h0f, h0b)
                if dbg:
                    nc.sync.dma_start(h0d.ap()[:, 0:TB], h0f[:])
                    nc.sync.dma_start(h0d.ap()[:, TB:2*TB], h0b[:])
                xw_precompute("1", chunks_for("1f", h0f, h0b),
                              chunks_for("1b", h0f, h0b))
                lstm_phase("1", h1f, h1b)

                wfc_a = lp.tile([H, NCLS], bf16)
                nc.sync.dma_start(wfc_a[:], wfc[0:H])
                wfc_b = lp.tile([H, NCLS], bf16)
                nc.sync.dma_start(wfc_b[:], wfc[H:])
                bfc_sb = lp.tile([BC, NCLS], f32)
                nc.sync.dma_start(bfc_sb[:], bfc[:])
                with tc.tile_pool(name="fcps", bufs=1, space="PSUM") as fps:
                    fc_ps = fps.tile([BC, NCLS], f32)
                    nc.tensor.matmul(fc_ps[:], lhsT=h1f[:, (T-1)*BC:T*BC],
                                     rhs=wfc_a[:], start=True, stop=False)
                    nc.tensor.matmul(fc_ps[:], lhsT=h1b[:, (T-1)*BC:T*BC],
                                     rhs=wfc_b[:], start=False, stop=True)
                    fc_sb = lp.tile([BC, NCLS], f32)
                    nc.vector.tensor_add(fc_sb[:], fc_ps[:], bfc_sb[:])
                    nc.sync.dma_start(out[:], fc_sb[:])
    return nc


# =====================================================================
# Entry
# =====================================================================

_CACHED = {}

def kernel(**inputs):
    xg_cores, idx_cores, S_cores = _prep_graph(
        inputs["x"], inputs["edge_src"], inputs["edge_dst"])
    wts = _prep_weights(inputs)

    if "nc" not in _CACHED:
        nc_new = build_kernel()
        if not nc_new.is_finalized():
            nc_new.finalize()
        _CACHED["nc"] = nc_new
    nc = _CACHED["nc"]

    in_maps = []
    for c in range(NCORES):
        m = dict(
            xg=xg_cores[c], idxT2=idx_cores[c],
            ST=S_cores[c][0], SSP=S_cores[c][1],
            identT=np.eye(128, dtype=np.float32).astype(BF16),
            W1=wts["W1"], W2=wts["W2"], W3=wts["W3"],
            b1=wts["b1"], b2=wts["b2"], b3=wts["b3"],
            wfc=wts["wfc"], bfc=wts["bfc"],
        )
        for tag in ("0", "1"):
            for dr in ("f", "b"):
                m[f"wih{tag}{dr}"] = wts[f"wih{tag}{dr}"]
                m[f"whh{tag}{dr}"] = wts[f"whh{tag}{dr}"]
        in_maps.append(m)

    res = run_bass_kernel_spmd(nc, in_maps, core_ids=list(range(NCORES)),
                               trace=os.environ.get("KTRACE", "0") == "1")
    kernel.last_result = res
    kernel.last_debug = res.results if os.environ.get("KDEBUG") == "1" else None
    outs = [res.results[c]["out"] for c in range(NCORES)]
    return np.concatenate(outs, axis=0).astype(np.float32)



# revision 15
# speedup vs baseline: 1.0441x; 1.0004x over previous
"""Trainium2 Bass kernel for GCN(3-layer) + BiLSTM(2-layer) + FC.

Self-contained: hardcodes all shapes; needs /opt/trn_rl_repo (concourse) only.

Architecture (8 NeuronCores, SPMD):
 - Nodes sharded by dst range (40960/core, graph-aligned).  A_hat = D^-1/2 (A+I) D^-1/2.
 - GCN layer 1: src features pre-gathered on host per (window, cell, slot) ->
   contiguous HWDGE loads, no indirect DMA.
 - GCN layers 2/3: ONE batched indirect gather per 512-node window (idx block
   [128, 43]), scatter-add via PE matmuls against host-built sparse cell
   blocks S (norm weights folded in), fused bias/relu on ACT, next-layer
   premultiply m_{k+1} = h_k @ W on PE, AllGather of m shards between layers.
 - GCN spill matmul accumulates directly into the cell-agg PSUM (start=True
   stop=False first, cells accumulate with start=False stop=True).
 - BiLSTM: batch-sharded 16 seq/core, feature-major state (H=128 partitions),
   per-step gate matmuls on PE, native Sigmoid/Tanh on ACT (same act-table
   set, no reloads), cell math on DVE; f/b run as two interleaved chains.
 - All contiguous DMA on SP HWDGE; GpSimd (SWDGE) reserved for the
   [128,1]-offset indirect gathers (the only reliable offset geometry on
   this bedrock env; see memory notes) and collectives.
"""
import sys, os
sys.path.insert(0, "/opt/trn_rl_repo")
import numpy as np
import ml_dtypes

import concourse.bass as bass
import concourse.bacc as bacc
import concourse.tile as tile
from concourse import mybir
from concourse.bass_utils import run_bass_kernel_spmd

BF16 = ml_dtypes.bfloat16
AF = mybir.ActivationFunctionType

# ---- problem constants (hardcoded) ----
B, T, FEAT, H, GCN_H, NCLS = 128, 512, 320, 128, 32, 10
N = B * 5 * T            # 327680
FIN = FEAT // 5          # 64
LSTM_IN = 5 * GCN_H      # 160
NCORES = 8
NS = N // NCORES         # 40960 nodes/core
WIN = 512                # window cols
NW = NS // WIN           # 80 windows/core
CELL = 16
NCELL = 32               # regular cells per window (16 cols each)
NCT = NCELL + 1          # + 1 spill cell (full-window S, accumulated last)
MAXE = 128               # max edges per cell slot-block
BC = B // NCORES         # 16 sequences per core
TB = T * BC              # 8192 token-cols per core
XWCH = 16                # lstm xw prefetch chunk (steps)


# =====================================================================
# Host preprocessing
# =====================================================================

def _prep_graph(x, edge_src, edge_dst):
    src = np.asarray(edge_src, np.int64)
    dst = np.asarray(edge_dst, np.int64)
    deg = np.bincount(dst, minlength=N).astype(np.float64) + 1.0
    dinv = 1.0 / np.sqrt(deg)
    sl = np.arange(N, dtype=np.int64)
    s_all = np.concatenate([src, sl])
    d_all = np.concatenate([dst, sl])
    w_all = (dinv[s_all] * dinv[d_all]).astype(np.float32)
    x_bf = np.asarray(x, np.float32).astype(BF16)

    # layers 2/3 gather-index remap: m rows are stored (window, p, c) packed
    vv = np.arange(N, dtype=np.int64)
    loc = vv % WIN
    M_REMAP = (vv // WIN) * WIN + (loc % 128) * 4 + loc // 128
    xg_cores, idx_cores, S_cores = [], [], []
    for c in range(NCORES):
        m = (d_all // NS) == c
        s_c = s_all[m]; d_c = d_all[m] - c * NS; w_c = w_all[m]
        wid = d_c // WIN
        cj = (d_c % WIN) // CELL
        cell_id = wid * NCELL + cj
        order = np.argsort(cell_id, kind="stable")
        s_c = s_c[order]; d_c = d_c[order]; w_c = w_c[order]
        cell_id = cell_id[order]
        counts = np.bincount(cell_id, minlength=NW * NCELL)
        starts = np.concatenate([[0], np.cumsum(counts)])
        rank = np.arange(len(s_c)) - starts[cell_id]
        wi = cell_id // NCELL
        ji = cell_id % NCELL
        reg = rank < MAXE
        # spill: per-window running slot for overflow edges
        sp = ~reg
        spill_slot = np.zeros(len(s_c), np.int64)
        ww = wi[sp]
        srt = np.argsort(ww, kind="stable")
        sslot = np.zeros(sp.sum(), np.int64)
        wcounts = np.bincount(ww, minlength=NW)
        wstarts = np.concatenate([[0], np.cumsum(wcounts)])
        sslot[srt] = np.arange(sp.sum()) - wstarts[ww[srt]]
        assert sp.sum() == 0 or sslot.max() < MAXE
        idx_arr = np.zeros((NW, NCT, MAXE), np.int32)
        idx_arr[wi[reg], ji[reg], rank[reg]] = s_c[reg]
        idx_arr[wi[sp], NCELL, sslot] = s_c[sp]
        # regular S: (MAXE, NW*512): cell j owns cols [w*512+j*16, +16)
        S_dev = np.zeros((MAXE, NW * WIN), np.float32)
        S_dev[rank[reg], wi[reg] * WIN + (d_c[reg] % WIN)] = w_c[reg]
        # spill S: (MAXE, NW*512) full-window routing for spill slots
        Ssp = np.zeros((MAXE, NW * WIN), np.float32)
        Ssp[sslot, wi[sp] * WIN + (d_c[sp] % WIN)] = w_c[sp]
        # layer-1 host pre-gather in (w, slot, cell) blocks
        xg = np.ascontiguousarray(
            x_bf[idx_arr].transpose(0, 2, 1, 3).reshape(NW, MAXE, NCT * FIN))
        xg_cores.append(xg)
        i1 = np.ascontiguousarray(
            idx_arr.transpose(2, 0, 1).reshape(MAXE, NW * NCT))
        i23 = M_REMAP[i1].astype(np.int32)
        idx_cores.append(i23)
        S_cores.append((S_dev.astype(BF16), Ssp.astype(BF16)))
    return xg_cores, idx_cores, S_cores


def _prep_weights(inp):
    d = {}
    d["W1"] = np.asarray(inp["W1"], np.float32).astype(BF16)
    d["W2"] = np.asarray(inp["W2"], np.float32).astype(BF16)
    d["W3"] = np.asarray(inp["W3"], np.float32).astype(BF16)
    for k in ("b1", "b2", "b3"):
        d[k] = np.asarray(inp[k], np.float32).reshape(GCN_H, 1)

    # gates use native ACT Sigmoid/Tanh: no weight pre-scaling
    GSCL = np.repeat([1.0, 1.0, 1.0, 1.0], H).astype(np.float32)

    def pack_ih(Wih, bih, bhh):
        Wih = np.asarray(Wih, np.float32)
        inn = Wih.shape[1]
        o = np.zeros((inn + 1, 4 * H), np.float32)
        o[:inn] = Wih.T
        o[inn] = np.asarray(bih, np.float32) + np.asarray(bhh, np.float32)
        return (o * GSCL).astype(BF16)

    def pack_hh(Whh):
        Whh = np.asarray(Whh, np.float32)
        return (np.concatenate(
            [Whh[g*H:(g+1)*H, :].T for g in range(4)], axis=1) * GSCL).astype(BF16)

    for tag in ("0", "1"):
        for dr in ("f", "b"):
            d[f"wih{tag}{dr}"] = pack_ih(
                inp[f"Wih{tag}{dr}"], inp[f"bih{tag}{dr}"], inp[f"bhh{tag}{dr}"])
            d[f"whh{tag}{dr}"] = pack_hh(inp[f"Whh{tag}{dr}"])
    d["wfc"] = np.asarray(inp["Wfc"], np.float32).astype(BF16)
    d["bfc"] = np.broadcast_to(
        np.asarray(inp["bfc"], np.float32), (BC, NCLS)).copy()
    return d


# =====================================================================
# Bass kernel builder
# =====================================================================

# torch gate order i,f,g,o -> device col blocks [i f o g]
GBLK = {0: 0, 1: 1, 2: 3, 3: 2}
USE_IDMM = os.environ.get("KIDMM", "0") == "1"
USE_GPS = os.environ.get("KGPS", "0") == "1"


def build_kernel():
    nc = bacc.Bacc(None, num_devices=NCORES)
    dt = mybir.dt
    f32, bf16, i32 = dt.float32, dt.bfloat16, dt.int32

    xg = nc.dram_tensor("xg", [NW, MAXE, NCT * FIN], bf16, kind="ExternalInput")
    idxT2 = nc.dram_tensor("idxT2", [MAXE, NW * NCT], i32, kind="ExternalInput")
    identT = nc.dram_tensor("identT", [128, 128], bf16, kind="ExternalInput")
    ST = nc.dram_tensor("ST", [MAXE, NW * WIN], bf16, kind="ExternalInput")
    SSP = nc.dram_tensor("SSP", [MAXE, NW * WIN], bf16, kind="ExternalInput")
    W1 = nc.dram_tensor("W1", [FIN, GCN_H], bf16, kind="ExternalInput")
    W2 = nc.dram_tensor("W2", [GCN_H, GCN_H], bf16, kind="ExternalInput")
    W3 = nc.dram_tensor("W3", [GCN_H, GCN_H], bf16, kind="ExternalInput")
    b1 = nc.dram_tensor("b1", [GCN_H, 1], f32, kind="ExternalInput")
    b2 = nc.dram_tensor("b2", [GCN_H, 1], f32, kind="ExternalInput")
    b3 = nc.dram_tensor("b3", [GCN_H, 1], f32, kind="ExternalInput")
    wih, whh = {}, {}
    for tag, inn in (("0", LSTM_IN), ("1", 2 * H)):
        for dr in ("f", "b"):
            wih[tag+dr] = nc.dram_tensor(f"wih{tag}{dr}", [inn + 1, 4 * H], bf16, kind="ExternalInput")
            whh[tag+dr] = nc.dram_tensor(f"whh{tag}{dr}", [H, 4 * H], bf16, kind="ExternalInput")
    wfc = nc.dram_tensor("wfc", [2 * H, NCLS], bf16, kind="ExternalInput")
    bfc = nc.dram_tensor("bfc", [BC, NCLS], f32, kind="ExternalInput")
    out = nc.dram_tensor("out", [BC, NCLS], f32, kind="ExternalOutput")

    dbg = os.environ.get("KDEBUG", "0") == "1"
    m2s = nc.dram_tensor("m2s", [NS, GCN_H], bf16)
    m3s = nc.dram_tensor("m3s", [NS, GCN_H], bf16)
    if dbg:
        m2d = nc.dram_tensor("m2d", [NS, GCN_H], bf16, kind="ExternalOutput")
        m3d = nc.dram_tensor("m3d", [NS, GCN_H], bf16, kind="ExternalOutput")
        gcn3d = nc.dram_tensor("gcn3d", [GCN_H, 5 * TB], bf16, kind="ExternalOutput")
        h0d = nc.dram_tensor("h0d", [H, 2 * TB], bf16, kind="ExternalOutput")
    m2f = nc.dram_tensor("m2f", [N, GCN_H], bf16, addr_space="Shared")
    m3f = nc.dram_tensor("m3f", [N, GCN_H], bf16, addr_space="Shared")
    # xw tensors: per phase, per dir: (H, T*4*BC): col = t*64 + blk*16 + b
    xwd = {ph+dr: nc.dram_tensor(f"xw{ph}{dr}", [H, T * 4 * BC], bf16)
           for ph in ("0", "1") for dr in ("f", "b")}

    RG = [list(range(NCORES))]

    with tile.TileContext(nc) as tc:
        with tc.tile_pool(name="persist", bufs=1) as pp:
            w1_sb = pp.tile([FIN, GCN_H], bf16)
            nc.sync.dma_start(w1_sb[:], W1[:])
            w2_sb = pp.tile([GCN_H, GCN_H], bf16)
            nc.sync.dma_start(w2_sb[:], W2[:])
            w3_sb = pp.tile([GCN_H, GCN_H], bf16)
            nc.sync.dma_start(w3_sb[:], W3[:])
            b_sb = {}
            for nm, t_ in (("b1", b1), ("b2", b2), ("b3", b3)):
                bias_tile = pp.tile([GCN_H, 1], f32, tag=nm)
                b_sb[nm] = bias_tile
                nc.sync.dma_start(b_sb[nm][:], t_[:])
            tokAll = pp.tile([GCN_H, 5 * TB], bf16)

            # ---------------- GCN ----------------
            def gcn_layer(lay, table, dst_shard, S_sb=None, idx_all=None):
                DIN = FIN if lay == 1 else GCN_H
                wnext = {1: w2_sb, 2: w3_sb}.get(lay)
                bias = b_sb[f"b{lay}"]
                nbuf = 2 if lay == 1 else 3
                with tc.tile_pool(name=f"gcn{lay}", bufs=nbuf) as gp, \
                     tc.tile_pool(name=f"gcnsp{lay}", bufs=2) as gsp, \
                     tc.tile_pool(name=f"gcnp{lay}", bufs=2, space="PSUM") as gps, \
                     tc.tile_pool(name=f"gcns{lay}", bufs=2) as sp2:
                    for w in range(NW):
                        g = gp.tile([MAXE, NCT * DIN], bf16, tag="g")
                        if lay == 1:
                            nc.sync.dma_start(g[:], table.ap()[w])
                        else:
                            for j in range(NCT):
                                nc.gpsimd.indirect_dma_start(
                                    out=g[:, j*DIN:(j+1)*DIN], out_offset=None,
                                    in_=table[:],
                                    in_offset=bass.IndirectOffsetOnAxis(
                                        ap=idx_all[:, w*NCT+j: w*NCT+j+1],
                                        axis=0),
                                )
                        ssp_sb = gsp.tile([MAXE, WIN], bf16, tag="ssp")
                        nc.sync.dma_start(ssp_sb[:], SSP[:, w*WIN:(w+1)*WIN])
                        agg = gps.tile([DIN, WIN], f32, tag="agg")
                        nc.tensor.matmul(
                            agg[:], lhsT=g[:, NCELL*DIN:NCT*DIN],
                            rhs=ssp_sb[:], start=True, stop=False)
                        for j in range(NCELL):
                            nc.tensor.matmul(
                                agg[:, j*CELL:(j+1)*CELL],
                                lhsT=g[:, j*DIN:(j+1)*DIN],
                                rhs=S_sb[:, w*WIN + j*CELL: w*WIN + (j+1)*CELL],
                                start=False, stop=True)
                        if lay == 1:
                            aggs = sp2.tile([FIN, WIN], bf16, tag="aggs")
                            nc.vector.tensor_copy(aggs[:], agg[:])
                            h_ps = gps.tile([GCN_H, WIN], f32, tag="hps")
                            nc.tensor.matmul(h_ps[:], lhsT=w1_sb[:], rhs=aggs[:],
                                             start=True, stop=True)
                            hT = sp2.tile([GCN_H, WIN], bf16, tag="hT")
                            nc.scalar.activation(hT[:], h_ps[:], AF.Relu, bias=bias[:])
                        elif lay == 2:
                            hT = sp2.tile([GCN_H, WIN], bf16, tag="hT")
                            nc.scalar.activation(hT[:], agg[:], AF.Relu, bias=bias[:])
                        else:
                            # write h3 directly into token layout:
                            # node v = c*NS + w*512 + loc; u=(v)%5; t=(v%2560)//5;
                            # b_w = w//5; token col = t*BC + b_w; tok row = 32u+f
                            bw = w // 5
                            a0w = (w % 5) * WIN
                            for j in range(5):
                                u = (a0w + j) % 5
                                t0 = (a0w + j) // 5
                                nj = (WIN - j + 4) // 5
                                srcv = agg[:, j:j + (nj - 1) * 5 + 1]
                                srcv.ap = mybir.VecI64Pair([list(srcv.ap[0]), [5, nj]])
                                c0 = u*TB + t0*BC + bw
                                dstv = tokAll[:, c0: c0 + (nj-1)*BC + 1]
                                dstv.ap = mybir.VecI64Pair([list(dstv.ap[0]), [BC, nj]])
                                nc.scalar.activation(dstv, srcv,
                                                     AF.Identity, bias=bias[:])
                        if dst_shard is not None:
                            mn_ps = gps.tile([MAXE, 4 * GCN_H], f32, tag="mnps")
                            for cc in range(4):
                                nc.tensor.matmul(
                                    mn_ps[:, cc*GCN_H:(cc+1)*GCN_H],
                                    lhsT=hT[:, cc*128:(cc+1)*128],
                                    rhs=wnext[:], start=True, stop=True)
                            mn = sp2.tile([MAXE, 4 * GCN_H], bf16, tag="mn")
                            nc.vector.tensor_copy(mn[:], mn_ps[:])
                            nc.sync.dma_start(
                                dst_shard.ap().rearrange(
                                    "(w p c) f -> w p (c f)", w=NW, p=MAXE)[w],
                                mn[:])

            with tc.tile_pool(name="spool", bufs=1) as spool:
                S_sb = spool.tile([MAXE, NW * WIN], bf16)
                nc.sync.dma_start(S_sb[:], ST[:])
                idx_all2 = spool.tile([MAXE, NW * NCT], i32)
                nc.sync.dma_start(idx_all2[:], idxT2[:])
                gcn_layer(1, xg, m2s, S_sb)
                nc.gpsimd.collective_compute(
                    "AllGather", mybir.AluOpType.bypass, replica_groups=RG,
                    ins=[m2s.ap().opt()], outs=[m2f.ap().opt()])
                gcn_layer(2, m2f, m3s, S_sb, idx_all2)
                nc.gpsimd.collective_compute(
                    "AllGather", mybir.AluOpType.bypass, replica_groups=RG,
                    ins=[m3s.ap().opt()], outs=[m3f.ap().opt()])
                gcn_layer(3, m3f, None, S_sb, idx_all2)
                if dbg:
                    nc.sync.dma_start(m2d[:], m2s[:])
                    nc.sync.dma_start(m3d[:], m3s[:])
                    nc.sync.dma_start(gcn3d[:], tokAll[:])

            # ---------------- LSTM ----------------
            with tc.tile_pool(name="lstm", bufs=1) as lp:
                h0f = lp.tile([H, TB], bf16)
                h0b = lp.tile([H, TB], bf16)
                h1f = lp.tile([H, TB], bf16)
                h1b = lp.tile([H, TB], bf16)
                ones1 = lp.tile([1, WIN], bf16)
                nc.vector.memset(ones1[:], 1.0)
                zeroBC = lp.tile([H, BC], bf16)
                nc.vector.memset(zeroBC[:], 0.0)
                ident_sb = lp.tile([128, 128], bf16)
                nc.sync.dma_start(ident_sb[:], identT[:])

                wih_sb = {}
                for key in ("0f", "0b"):
                    wchunks = lp.tile([GCN_H, 5 * 4 * H], bf16, tag=f"wih5{key}",
                                      name=f"wih5{key}")
                    for u in range(5):
                        nc.sync.dma_start(
                            wchunks[:, u*4*H:(u+1)*4*H], wih[key][u*32:(u+1)*32])
                    wihBias = lp.tile([1, 4 * H], bf16, tag=f"wihC{key}",
                                      name=f"wihC{key}")
                    nc.sync.dma_start(wihBias[:], wih[key][LSTM_IN:LSTM_IN+1])
                    wih_sb[key] = (wchunks, wihBias)
                for key in ("1f", "1b"):
                    wihA = lp.tile([128, 4 * H], bf16, tag=f"wihA{key}", name=f"wihA{key}")
                    nc.sync.dma_start(wihA[:], wih[key][0:128])
                    wihB = lp.tile([128, 4 * H], bf16, tag=f"wihB{key}", name=f"wihB{key}")
                    nc.sync.dma_start(wihB[:], wih[key][128:256])
                    wihBias = lp.tile([1, 4 * H], bf16, tag=f"wihC{key}", name=f"wihC{key}")
                    nc.sync.dma_start(wihBias[:], wih[key][256:257])
                    wih_sb[key] = (wihA, wihB, wihBias)
                whh_sb = {}
                for key in ("0f", "0b", "1f", "1b"):
                    whhT = lp.tile([H, 4 * H], bf16, tag=f"whh{key}")
                    nc.sync.dma_start(whhT[:], whh[key][:])
                    whh_sb[key] = whhT

                def xw_precompute(ph, chunks_f, chunks_b):
                    # chunks: per dir list of (wih_rows_slice, rhs_ap) K-chunks
                    with tc.tile_pool(name=f"xw{ph}", bufs=3) as xp, \
                         tc.tile_pool(name=f"xwp{ph}", bufs=2, space="PSUM") as xps:
                        for dr, chunks in (("f", chunks_f), ("b", chunks_b)):
                            for G in range(4):
                                blk = GBLK[G]
                                for ct in range(TB // WIN):
                                    ps = xps.tile([H, WIN], f32, tag="ps")
                                    nchunks = len(chunks)
                                    for ci, (lhsT, rhs) in enumerate(chunks):
                                        rr = (rhs[:, 0:WIN] if rhs.shape[0] == 1
                                              else rhs[:, ct*WIN:(ct+1)*WIN])
                                        nc.tensor.matmul(
                                            ps[:], lhsT=lhsT[:, G*H:(G+1)*H],
                                            rhs=rr,
                                            start=(ci == 0), stop=(ci == nchunks-1))
                                    sb = xp.tile([H, WIN], bf16, tag="sb")
                                    nc.vector.tensor_copy(sb[:], ps[:])
                                    # layout (g, t, b): gate block contiguous
                                    nc.sync.dma_start(
                                        xwd[ph+dr].ap()[:, blk*TB + ct*WIN: blk*TB + (ct+1)*WIN],
                                        sb[:])

                def lstm_phase(ph, hf_st, hb_st):
                    # split f/b chains; gates scaled so sigmoid(x)=0.5+0.5*tanh(x)
                    # (0.5 factor folded into i,f,o weights at host). Gate block
                    # order in PSUM: [i f o g].
                    with tc.tile_pool(name=f"lph{ph}", bufs=3) as php, \
                         tc.tile_pool(name=f"lxw{ph}", bufs=2) as pxw, \
                         tc.tile_pool(name=f"lpc{ph}", bufs=1) as phc, \
                         tc.tile_pool(name=f"lps{ph}", bufs=4, space="PSUM") as phps:
                        cst = {}
                        for dr in ("f", "b"):
                            cst[dr] = phc.tile([H, BC], f32, tag=f"c{dr}", name=f"cst{dr}")
                            nc.vector.memset(cst[dr][:], 0.0)
                        halfs = phc.tile([H, 3 * BC], f32, tag="halfs")
                        nc.vector.memset(halfs[:], 0.5)
                        xw_b = {}
                        for t in range(T):
                            tt = T - 1 - t
                            if t % XWCH == 0:
                                for dr in ("f", "b"):
                                    xw_b[dr] = pxw.tile([H, 4, XWCH * BC], bf16, tag=f"xw{dr}", name=f"xwb{dr}")
                                    base = t if dr == "f" else tt - XWCH + 1
                                    for Gb in range(4):
                                        nc.sync.dma_start(
                                            xw_b[dr][:, Gb, :],
                                            xwd[ph+dr].ap()[:, Gb*TB + base*BC: Gb*TB + (base+XWCH)*BC])
                            of = (t % XWCH) * BC
                            ob = (XWCH - 1 - (t % XWCH)) * BC
                            gate_ps, sact = {}, {}
                            for d_i, dr in ((0, "f"), (1, "b")):
                                st = hf_st if d_i == 0 else hb_st
                                tm = t if d_i == 0 else tt
                                if t == 0:
                                    hprev = zeroBC[:]
                                elif d_i == 0:
                                    hprev = st[:, (tm-1)*BC:tm*BC]
                                else:
                                    hprev = st[:, (tm+1)*BC:(tm+2)*BC]
                                wt = whh_sb[ph + dr]
                                gp_ = phps.tile([H, 4 * BC], f32, tag=f"g{dr}", name=f"gps{dr}")
                                gate_ps[dr] = gp_
                                off = of if d_i == 0 else ob
                                if USE_IDMM:
                                    for G in range(4):
                                        blk = GBLK[G]
                                        nc.tensor.matmul(
                                            gp_[:, blk*BC:(blk+1)*BC],
                                            lhsT=wt[:, G*H:(G+1)*H], rhs=hprev,
                                            start=True, stop=False)
                                    nc.tensor.matmul(
                                        gp_[:], lhsT=ident_sb[:],
                                        rhs=xw_b[dr][:, :, off:off+BC],
                                        start=False, stop=True)
                                else:
                                    for G in range(4):
                                        blk = GBLK[G]
                                        nc.tensor.matmul(
                                            gp_[:, blk*BC:(blk+1)*BC],
                                            lhsT=wt[:, G*H:(G+1)*H], rhs=hprev,
                                            start=True, stop=True)
                            for dr in ("f", "b"):
                                sa = php.tile([H, 4 * BC], f32, tag=f"sa{dr}", name=f"sa{dr}")
                                sact[dr] = sa
                                off = of if dr == "f" else ob
                                gf_ = php.tile([H, 4 * BC], f32, tag=f"gf{dr}", name=f"gf{dr}")
                                nc.vector.tensor_add(
                                    gf_[:].rearrange("h (g b) -> h g b", g=4),
                                    gate_ps[dr][:].rearrange("h (g b) -> h g b", g=4),
                                    xw_b[dr][:, :, off:off+BC])
                                nc.scalar.activation(sa[:, 0:3*BC], gf_[:, 0:3*BC],
                                                     AF.Sigmoid)
                                nc.scalar.activation(sa[:, 3*BC:4*BC],
                                                     gf_[:, 3*BC:4*BC], AF.Tanh)
                            mm = {}
                            for dr in ("f", "b"):
                                sa = sact[dr]
                                mm[dr] = php.tile([H, BC], f32, tag=f"mm{dr}", name=f"mmt{dr}")
                                nc.vector.tensor_mul(mm[dr][:], sa[:, 0:BC], sa[:, 3*BC:4*BC])
                            eng = nc.gpsimd if USE_GPS else nc.vector
                            for dr in ("f", "b"):
                                eng.tensor_mul(cst[dr][:], cst[dr][:], sact[dr][:, BC:2*BC])
                            for dr in ("f", "b"):
                                eng.tensor_add(cst[dr][:], cst[dr][:], mm[dr][:])
                            tc_ = {}
                            for dr in ("f", "b"):
                                tc_[dr] = php.tile([H, BC], f32, tag=f"tc{dr}", name=f"tct{dr}")
                                nc.scalar.activation(tc_[dr][:], cst[dr][:], AF.Tanh)
                            nc.vector.tensor_mul(
                                hf_st[:, t*BC:(t+1)*BC], sact["f"][:, 2*BC:3*BC], tc_["f"][:])
                            nc.vector.tensor_mul(
                                hb_st[:, tt*BC:(tt+1)*BC], sact["b"][:, 2*BC:3*BC], tc_["b"][:])

                def chunks0(key):
                    wchunks, cbias = wih_sb[key]
                    ch = [(wchunks[:, u*4*H:(u+1)*4*H], tokAll[:, u*TB:(u+1)*TB])
                          for u in range(5)]
                    ch.append((cbias, ones1))
                    return ch

                def chunks_for(key, rA, rB):
                    a, bwt, cbias = wih_sb[key]
                    return [(a, rA), (bwt, rB), (cbias, ones1)]
                xw_precompute("0", chunks0("0f"), chunks0("0b"))
                lstm_phase("0", h0f, h0b)
                if dbg:
                    nc.sync.dma_start(h0d.ap()[:, 0:TB], h0f[:])
                    nc.sync.dma_start(h0d.ap()[:, TB:2*TB], h0b[:])
                xw_precompute("1", chunks_for("1f", h0f, h0b),
                              chunks_for("1b", h0f, h0b))
                lstm_phase("1", h1f, h1b)

                wfc_a = lp.tile([H, NCLS], bf16)
                nc.sync.dma_start(wfc_a[:], wfc[0:H])
                wfc_b = lp.tile([H, NCLS], bf16)
                nc.sync.dma_start(wfc_b[:], wfc[H:])
                bfc_sb = lp.tile([BC, NCLS], f32)
                nc.sync.dma_start(bfc_sb[:], bfc[:])
                with tc.tile_pool(name="fcps", bufs=1, space="PSUM") as fps:
                    fc_ps = fps.tile([BC, NCLS], f32)
                    nc.tensor.matmul(fc_ps[:], lhsT=h1f[:, (T-1)*BC:T*BC],
                                     rhs=wfc_a[:], start=True, stop=False)
                    nc.tensor.matmul(fc_ps[:], lhsT=h1b[:, (T-1)*BC:T*BC],
                                     rhs=wfc_b[:], start=False, stop=True)
                    fc_sb = lp.tile([BC, NCLS], f32)
                    nc.vector.tensor_add(fc_sb[:], fc_ps[:], bfc_sb[:])
                    nc.sync.dma_start(out[:], fc_sb[:])
    return nc


# =====================================================================
# Entry
# =====================================================================

_CACHED = {}

def kernel(**inputs):
    xg_cores, idx_cores, S_cores = _prep_graph(
        inputs["x"], inputs["edge_src"], inputs["edge_dst"])
    wts = _prep_weights(inputs)

    if "nc" not in _CACHED:
        nc_new = build_kernel()
        if not nc_new.is_finalized():
            nc_new.finalize()
        _CACHED["nc"] = nc_new
    nc = _CACHED["nc"]

    in_maps = []
    for c in range(NCORES):
        m = dict(
            xg=xg_cores[c], idxT2=idx_cores[c],
            ST=S_cores[c][0], SSP=S_cores[c][1],
            identT=np.eye(128, dtype=np.float32).astype(BF16),
            W1=wts["W1"], W2=wts["W2"], W3=wts["W3"],
            b1=wts["b1"], b2=wts["b2"], b3=wts["b3"],
            wfc=wts["wfc"], bfc=wts["bfc"],
        )
        for tag in ("0", "1"):
            for dr in ("f", "b"):
                m[f"wih{tag}{dr}"] = wts[f"wih{tag}{dr}"]
                m[f"whh{tag}{dr}"] = wts[f"whh{tag}{dr}"]
        in_maps.append(m)

    res = run_bass_kernel_spmd(nc, in_maps, core_ids=list(range(NCORES)),
                               trace=os.environ.get("KTRACE", "0") == "1")
    kernel.last_result = res
    kernel.last_debug = res.results if os.environ.get("KDEBUG") == "1" else None
    outs = [res.results[c]["out"] for c in range(NCORES)]
    return np.concatenate(outs, axis=0).astype(np.float32)



# revision 19
# speedup vs baseline: 1.0882x; 1.0423x over previous
"""Trainium2 Bass kernel for GCN(3-layer) + BiLSTM(2-layer) + FC.

Self-contained: hardcodes all shapes; needs /opt/trn_rl_repo (concourse) only.

Architecture (8 NeuronCores, SPMD):
 - Nodes sharded by dst range (40960/core, graph-aligned).  A_hat = D^-1/2 (A+I) D^-1/2.
 - GCN layer 1: src features pre-gathered on host per (window, cell, slot) ->
   contiguous HWDGE loads, no indirect DMA.
 - GCN layers 2/3: ONE batched indirect gather per 512-node window (idx block
   [128, 43]), scatter-add via PE matmuls against host-built sparse cell
   blocks S (norm weights folded in), fused bias/relu on ACT, next-layer
   premultiply m_{k+1} = h_k @ W on PE, AllGather of m shards between layers.
 - GCN spill matmul accumulates directly into the cell-agg PSUM (start=True
   stop=False first, cells accumulate with start=False stop=True).
 - BiLSTM: batch-sharded 16 seq/core, feature-major state (H=128 partitions),
   per-step gate matmuls on PE, native Sigmoid/Tanh on ACT (same act-table
   set, no reloads), cell math on DVE; f/b run as two interleaved chains.
 - All contiguous DMA on SP HWDGE; GpSimd (SWDGE) reserved for the
   [128,1]-offset indirect gathers (the only reliable offset geometry on
   this bedrock env; see memory notes) and collectives.
"""
import sys, os
sys.path.insert(0, "/opt/trn_rl_repo")
import numpy as np
import ml_dtypes

import concourse.bass as bass
import concourse.bacc as bacc
import concourse.tile as tile
from concourse import mybir
from concourse.bass_utils import run_bass_kernel_spmd

BF16 = ml_dtypes.bfloat16
AF = mybir.ActivationFunctionType

# ---- problem constants (hardcoded) ----
B, T, FEAT, H, GCN_H, NCLS = 128, 512, 320, 128, 32, 10
N = B * 5 * T            # 327680
FIN = FEAT // 5          # 64
LSTM_IN = 5 * GCN_H      # 160
NCORES = 8
NS = N // NCORES         # 40960 nodes/core
WIN = 512                # window cols
NW = NS // WIN           # 80 windows/core
CELL = 16
NCELL = 32               # regular cells per window (16 cols each)
NCT = NCELL + 1          # + 1 spill cell (full-window S, accumulated last)
MAXE = 128               # max edges per cell slot-block
BC = B // NCORES         # 16 sequences per core
TB = T * BC              # 8192 token-cols per core
XWCH = 16                # lstm xw prefetch chunk (steps)


# =====================================================================
# Host preprocessing
# =====================================================================

def _prep_graph(x, edge_src, edge_dst):
    src = np.asarray(edge_src, np.int64)
    dst = np.asarray(edge_dst, np.int64)
    deg = np.bincount(dst, minlength=N).astype(np.float64) + 1.0
    dinv = 1.0 / np.sqrt(deg)
    sl = np.arange(N, dtype=np.int64)
    s_all = np.concatenate([src, sl])
    d_all = np.concatenate([dst, sl])
    w_all = (dinv[s_all] * dinv[d_all]).astype(np.float32)
    x_bf = np.asarray(x, np.float32).astype(BF16)

    # layers 2/3 gather-index remap: m rows are stored (window, p, c) packed
    vv = np.arange(N, dtype=np.int64)
    loc = vv % WIN
    M_REMAP = (vv // WIN) * WIN + (loc % 128) * 4 + loc // 128
    xg_cores, idx_cores, S_cores = [], [], []
    for c in range(NCORES):
        m = (d_all // NS) == c
        s_c = s_all[m]; d_c = d_all[m] - c * NS; w_c = w_all[m]
        wid = d_c // WIN
        cj = (d_c % WIN) // CELL
        cell_id = wid * NCELL + cj
        order = np.argsort(cell_id, kind="stable")
        s_c = s_c[order]; d_c = d_c[order]; w_c = w_c[order]
        cell_id = cell_id[order]
        counts = np.bincount(cell_id, minlength=NW * NCELL)
        starts = np.concatenate([[0], np.cumsum(counts)])
        rank = np.arange(len(s_c)) - starts[cell_id]
        wi = cell_id // NCELL
        ji = cell_id % NCELL
        reg = rank < MAXE
        # spill: per-window running slot for overflow edges
        sp = ~reg
        spill_slot = np.zeros(len(s_c), np.int64)
        ww = wi[sp]
        srt = np.argsort(ww, kind="stable")
        sslot = np.zeros(sp.sum(), np.int64)
        wcounts = np.bincount(ww, minlength=NW)
        wstarts = np.concatenate([[0], np.cumsum(wcounts)])
        sslot[srt] = np.arange(sp.sum()) - wstarts[ww[srt]]
        assert sp.sum() == 0 or sslot.max() < MAXE
        idx_arr = np.zeros((NW, NCT, MAXE), np.int32)
        idx_arr[wi[reg], ji[reg], rank[reg]] = s_c[reg]
        idx_arr[wi[sp], NCELL, sslot] = s_c[sp]
        # regular S: (MAXE, NW*512): cell j owns cols [w*512+j*16, +16)
        S_dev = np.zeros((MAXE, NW * WIN), np.float32)
        S_dev[rank[reg], wi[reg] * WIN + (d_c[reg] % WIN)] = w_c[reg]
        # spill S: (MAXE, NW*512) full-window routing for spill slots
        Ssp = np.zeros((MAXE, NW * WIN), np.float32)
        Ssp[sslot, wi[sp] * WIN + (d_c[sp] % WIN)] = w_c[sp]
        # layer-1 host pre-gather in (w, slot, cell) blocks
        xg = np.ascontiguousarray(
            x_bf[idx_arr].transpose(0, 2, 1, 3).reshape(NW, MAXE, NCT * FIN))
        xg_cores.append(xg)
        i1 = np.ascontiguousarray(
            idx_arr.transpose(2, 0, 1).reshape(MAXE, NW * NCT))
        i23 = M_REMAP[i1].astype(np.int32)
        idx_cores.append(i23)
        S_cores.append((S_dev.astype(BF16), Ssp.astype(BF16)))
    return xg_cores, idx_cores, S_cores


def _prep_weights(inp):
    d = {}
    d["W1"] = np.asarray(inp["W1"], np.float32).astype(BF16)
    d["W2"] = np.asarray(inp["W2"], np.float32).astype(BF16)
    d["W3"] = np.asarray(inp["W3"], np.float32).astype(BF16)
    for k in ("b1", "b2", "b3"):
        d[k] = np.asarray(inp[k], np.float32).reshape(GCN_H, 1)

    # gates use native ACT Sigmoid/Tanh: no weight pre-scaling
    GSCL = np.repeat([1.0, 1.0, 1.0, 1.0], H).astype(np.float32)

    def pack_ih(Wih, bih, bhh):
        Wih = np.asarray(Wih, np.float32)
        inn = Wih.shape[1]
        o = np.zeros((inn + 1, 4 * H), np.float32)
        o[:inn] = Wih.T
        o[inn] = np.asarray(bih, np.float32) + np.asarray(bhh, np.float32)
        return (o * GSCL).astype(BF16)

    def pack_hh(Whh):
        Whh = np.asarray(Whh, np.float32)
        return (np.concatenate(
            [Whh[g*H:(g+1)*H, :].T for g in range(4)], axis=1) * GSCL).astype(BF16)

    for tag in ("0", "1"):
        for dr in ("f", "b"):
            d[f"wih{tag}{dr}"] = pack_ih(
                inp[f"Wih{tag}{dr}"], inp[f"bih{tag}{dr}"], inp[f"bhh{tag}{dr}"])
            d[f"whh{tag}{dr}"] = pack_hh(inp[f"Whh{tag}{dr}"])
    d["wfc"] = np.asarray(inp["Wfc"], np.float32).astype(BF16)
    d["bfc"] = np.broadcast_to(
        np.asarray(inp["bfc"], np.float32), (BC, NCLS)).copy()
    return d


# =====================================================================
# Bass kernel builder
# =====================================================================

# torch gate order i,f,g,o -> device col blocks [i f o g]
GBLK = {0: 0, 1: 1, 2: 3, 3: 2}
USE_IDMM = os.environ.get("KIDMM", "1") == "1"
USE_GPS = os.environ.get("KGPS", "0") == "1"


def build_kernel():
    nc = bacc.Bacc(None, num_devices=NCORES)
    dt = mybir.dt
    f32, bf16, i32 = dt.float32, dt.bfloat16, dt.int32

    xg = nc.dram_tensor("xg", [NW, MAXE, NCT * FIN], bf16, kind="ExternalInput")
    idxT2 = nc.dram_tensor("idxT2", [MAXE, NW * NCT], i32, kind="ExternalInput")
    identT = nc.dram_tensor("identT", [128, 128], bf16, kind="ExternalInput")
    ST = nc.dram_tensor("ST", [MAXE, NW * WIN], bf16, kind="ExternalInput")
    SSP = nc.dram_tensor("SSP", [MAXE, NW * WIN], bf16, kind="ExternalInput")
    W1 = nc.dram_tensor("W1", [FIN, GCN_H], bf16, kind="ExternalInput")
    W2 = nc.dram_tensor("W2", [GCN_H, GCN_H], bf16, kind="ExternalInput")
    W3 = nc.dram_tensor("W3", [GCN_H, GCN_H], bf16, kind="ExternalInput")
    b1 = nc.dram_tensor("b1", [GCN_H, 1], f32, kind="ExternalInput")
    b2 = nc.dram_tensor("b2", [GCN_H, 1], f32, kind="ExternalInput")
    b3 = nc.dram_tensor("b3", [GCN_H, 1], f32, kind="ExternalInput")
    wih, whh = {}, {}
    for tag, inn in (("0", LSTM_IN), ("1", 2 * H)):
        for dr in ("f", "b"):
            wih[tag+dr] = nc.dram_tensor(f"wih{tag}{dr}", [inn + 1, 4 * H], bf16, kind="ExternalInput")
            whh[tag+dr] = nc.dram_tensor(f"whh{tag}{dr}", [H, 4 * H], bf16, kind="ExternalInput")
    wfc = nc.dram_tensor("wfc", [2 * H, NCLS], bf16, kind="ExternalInput")
    bfc = nc.dram_tensor("bfc", [BC, NCLS], f32, kind="ExternalInput")
    out = nc.dram_tensor("out", [BC, NCLS], f32, kind="ExternalOutput")

    dbg = os.environ.get("KDEBUG", "0") == "1"
    m2s = nc.dram_tensor("m2s", [NS, GCN_H], bf16)
    m3s = nc.dram_tensor("m3s", [NS, GCN_H], bf16)
    if dbg:
        m2d = nc.dram_tensor("m2d", [NS, GCN_H], bf16, kind="ExternalOutput")
        m3d = nc.dram_tensor("m3d", [NS, GCN_H], bf16, kind="ExternalOutput")
        gcn3d = nc.dram_tensor("gcn3d", [GCN_H, 5 * TB], bf16, kind="ExternalOutput")
        h0d = nc.dram_tensor("h0d", [H, 2 * TB], bf16, kind="ExternalOutput")
    m2f = nc.dram_tensor("m2f", [N, GCN_H], bf16, addr_space="Shared")
    m3f = nc.dram_tensor("m3f", [N, GCN_H], bf16, addr_space="Shared")
    # xw tensors: per phase, per dir: (H, T*4*BC): col = t*64 + blk*16 + b
    xwd = {ph+dr: nc.dram_tensor(f"xw{ph}{dr}", [H, T * 4 * BC], bf16)
           for ph in ("0", "1") for dr in ("f", "b")}

    RG = [list(range(NCORES))]

    with tile.TileContext(nc) as tc:
        with tc.tile_pool(name="persist", bufs=1) as pp:
            w1_sb = pp.tile([FIN, GCN_H], bf16)
            nc.sync.dma_start(w1_sb[:], W1[:])
            w2_sb = pp.tile([GCN_H, GCN_H], bf16)
            nc.sync.dma_start(w2_sb[:], W2[:])
            w3_sb = pp.tile([GCN_H, GCN_H], bf16)
            nc.sync.dma_start(w3_sb[:], W3[:])
            b_sb = {}
            for nm, t_ in (("b1", b1), ("b2", b2), ("b3", b3)):
                bias_tile = pp.tile([GCN_H, 1], f32, tag=nm)
                b_sb[nm] = bias_tile
                nc.sync.dma_start(b_sb[nm][:], t_[:])
            tokAll = pp.tile([GCN_H, 5 * TB], bf16)

            # ---------------- GCN ----------------
            def gcn_layer(lay, table, dst_shard, S_sb=None, idx_all=None):
                DIN = FIN if lay == 1 else GCN_H
                wnext = {1: w2_sb, 2: w3_sb}.get(lay)
                bias = b_sb[f"b{lay}"]
                nbuf = 2 if lay == 1 else 3
                with tc.tile_pool(name=f"gcn{lay}", bufs=nbuf) as gp, \
                     tc.tile_pool(name=f"gcnsp{lay}", bufs=2) as gsp, \
                     tc.tile_pool(name=f"gcnp{lay}", bufs=2, space="PSUM") as gps, \
                     tc.tile_pool(name=f"gcns{lay}", bufs=2) as sp2:
                    for w in range(NW):
                        g = gp.tile([MAXE, NCT * DIN], bf16, tag="g")
                        if lay == 1:
                            nc.sync.dma_start(g[:], table.ap()[w])
                        else:
                            for j in range(NCT):
                                nc.gpsimd.indirect_dma_start(
                                    out=g[:, j*DIN:(j+1)*DIN], out_offset=None,
                                    in_=table[:],
                                    in_offset=bass.IndirectOffsetOnAxis(
                                        ap=idx_all[:, w*NCT+j: w*NCT+j+1],
                                        axis=0),
                                )
                        ssp_sb = gsp.tile([MAXE, WIN], bf16, tag="ssp")
                        nc.sync.dma_start(ssp_sb[:], SSP[:, w*WIN:(w+1)*WIN])
                        agg = gps.tile([DIN, WIN], f32, tag="agg")
                        nc.tensor.matmul(
                            agg[:], lhsT=g[:, NCELL*DIN:NCT*DIN],
                            rhs=ssp_sb[:], start=True, stop=False)
                        for j in range(NCELL):
                            nc.tensor.matmul(
                                agg[:, j*CELL:(j+1)*CELL],
                                lhsT=g[:, j*DIN:(j+1)*DIN],
                                rhs=S_sb[:, w*WIN + j*CELL: w*WIN + (j+1)*CELL],
                                start=False, stop=True)
                        if lay == 1:
                            aggs = sp2.tile([FIN, WIN], bf16, tag="aggs")
                            nc.vector.tensor_copy(aggs[:], agg[:])
                            h_ps = gps.tile([GCN_H, WIN], f32, tag="hps")
                            nc.tensor.matmul(h_ps[:], lhsT=w1_sb[:], rhs=aggs[:],
                                             start=True, stop=True)
                            hT = sp2.tile([GCN_H, WIN], bf16, tag="hT")
                            nc.scalar.activation(hT[:], h_ps[:], AF.Relu, bias=bias[:])
                        elif lay == 2:
                            hT = sp2.tile([GCN_H, WIN], bf16, tag="hT")
                            nc.scalar.activation(hT[:], agg[:], AF.Relu, bias=bias[:])
                        else:
                            # write h3 directly into token layout:
                            # node v = c*NS + w*512 + loc; u=(v)%5; t=(v%2560)//5;
                            # b_w = w//5; token col = t*BC + b_w; tok row = 32u+f
                            bw = w // 5
                            a0w = (w % 5) * WIN
                            for j in range(5):
                                u = (a0w + j) % 5
                                t0 = (a0w + j) // 5
                                nj = (WIN - j + 4) // 5
                                srcv = agg[:, j:j + (nj - 1) * 5 + 1]
                                srcv.ap = mybir.VecI64Pair([list(srcv.ap[0]), [5, nj]])
                                c0 = u*TB + t0*BC + bw
                                dstv = tokAll[:, c0: c0 + (nj-1)*BC + 1]
                                dstv.ap = mybir.VecI64Pair([list(dstv.ap[0]), [BC, nj]])
                                nc.scalar.activation(dstv, srcv,
                                                     AF.Identity, bias=bias[:])
                        if dst_shard is not None:
                            mn_ps = gps.tile([MAXE, 4 * GCN_H], f32, tag="mnps")
                            for cc in range(4):
                                nc.tensor.matmul(
                                    mn_ps[:, cc*GCN_H:(cc+1)*GCN_H],
                                    lhsT=hT[:, cc*128:(cc+1)*128],
                                    rhs=wnext[:], start=True, stop=True)
                            mn = sp2.tile([MAXE, 4 * GCN_H], bf16, tag="mn")
                            nc.vector.tensor_copy(mn[:], mn_ps[:])
                            nc.sync.dma_start(
                                dst_shard.ap().rearrange(
                                    "(w p c) f -> w p (c f)", w=NW, p=MAXE)[w],
                                mn[:])

            with tc.tile_pool(name="spool", bufs=1) as spool:
                S_sb = spool.tile([MAXE, NW * WIN], bf16)
                nc.sync.dma_start(S_sb[:], ST[:])
                idx_all2 = spool.tile([MAXE, NW * NCT], i32)
                nc.sync.dma_start(idx_all2[:], idxT2[:])
                gcn_layer(1, xg, m2s, S_sb)
                nc.gpsimd.collective_compute(
                    "AllGather", mybir.AluOpType.bypass, replica_groups=RG,
                    ins=[m2s.ap().opt()], outs=[m2f.ap().opt()])
                gcn_layer(2, m2f, m3s, S_sb, idx_all2)
                nc.gpsimd.collective_compute(
                    "AllGather", mybir.AluOpType.bypass, replica_groups=RG,
                    ins=[m3s.ap().opt()], outs=[m3f.ap().opt()])
                gcn_layer(3, m3f, None, S_sb, idx_all2)
                if dbg:
                    nc.sync.dma_start(m2d[:], m2s[:])
                    nc.sync.dma_start(m3d[:], m3s[:])
                    nc.sync.dma_start(gcn3d[:], tokAll[:])

            # ---------------- LSTM ----------------
            with tc.tile_pool(name="lstm", bufs=1) as lp:
                h0f = lp.tile([H, TB], bf16)
                h0b = lp.tile([H, TB], bf16)
                h1f = lp.tile([H, TB], bf16)
                h1b = lp.tile([H, TB], bf16)
                ones1 = lp.tile([1, WIN], bf16)
                nc.vector.memset(ones1[:], 1.0)
                zeroBC = lp.tile([H, BC], bf16)
                nc.vector.memset(zeroBC[:], 0.0)
                ident_sb = lp.tile([128, 128], bf16)
                nc.sync.dma_start(ident_sb[:], identT[:])

                wih_sb = {}
                for key in ("0f", "0b"):
                    wchunks = lp.tile([GCN_H, 5 * 4 * H], bf16, tag=f"wih5{key}",
                                      name=f"wih5{key}")
                    for u in range(5):
                        nc.sync.dma_start(
                            wchunks[:, u*4*H:(u+1)*4*H], wih[key][u*32:(u+1)*32])
                    wihBias = lp.tile([1, 4 * H], bf16, tag=f"wihC{key}",
                                      name=f"wihC{key}")
                    nc.sync.dma_start(wihBias[:], wih[key][LSTM_IN:LSTM_IN+1])
                    wih_sb[key] = (wchunks, wihBias)
                for key in ("1f", "1b"):
                    wihA = lp.tile([128, 4 * H], bf16, tag=f"wihA{key}", name=f"wihA{key}")
                    nc.sync.dma_start(wihA[:], wih[key][0:128])
                    wihB = lp.tile([128, 4 * H], bf16, tag=f"wihB{key}", name=f"wihB{key}")
                    nc.sync.dma_start(wihB[:], wih[key][128:256])
                    wihBias = lp.tile([1, 4 * H], bf16, tag=f"wihC{key}", name=f"wihC{key}")
                    nc.sync.dma_start(wihBias[:], wih[key][256:257])
                    wih_sb[key] = (wihA, wihB, wihBias)
                whh_sb = {}
                for key in ("0f", "0b", "1f", "1b"):
                    whhT = lp.tile([H, 4 * H], bf16, tag=f"whh{key}")
                    nc.sync.dma_start(whhT[:], whh[key][:])
                    whh_sb[key] = whhT

                def xw_precompute(ph, chunks_f, chunks_b):
                    # chunks: per dir list of (wih_rows_slice, rhs_ap) K-chunks
                    with tc.tile_pool(name=f"xw{ph}", bufs=3) as xp, \
                         tc.tile_pool(name=f"xwp{ph}", bufs=2, space="PSUM") as xps:
                        for dr, chunks in (("f", chunks_f), ("b", chunks_b)):
                            for G in range(4):
                                blk = GBLK[G]
                                for ct in range(TB // WIN):
                                    ps = xps.tile([H, WIN], f32, tag="ps")
                                    nchunks = len(chunks)
                                    for ci, (lhsT, rhs) in enumerate(chunks):
                                        rr = (rhs[:, 0:WIN] if rhs.shape[0] == 1
                                              else rhs[:, ct*WIN:(ct+1)*WIN])
                                        nc.tensor.matmul(
                                            ps[:], lhsT=lhsT[:, G*H:(G+1)*H],
                                            rhs=rr,
                                            start=(ci == 0), stop=(ci == nchunks-1))
                                    sb = xp.tile([H, WIN], bf16, tag="sb")
                                    nc.vector.tensor_copy(sb[:], ps[:])
                                    # layout (g, t, b): gate block contiguous
                                    nc.sync.dma_start(
                                        xwd[ph+dr].ap()[:, blk*TB + ct*WIN: blk*TB + (ct+1)*WIN],
                                        sb[:])

                def lstm_phase(ph, hf_st, hb_st):
                    # split f/b chains; gates scaled so sigmoid(x)=0.5+0.5*tanh(x)
                    # (0.5 factor folded into i,f,o weights at host). Gate block
                    # order in PSUM: [i f o g].
                    with tc.tile_pool(name=f"lph{ph}", bufs=3) as php, \
                         tc.tile_pool(name=f"lxw{ph}", bufs=2) as pxw, \
                         tc.tile_pool(name=f"lpc{ph}", bufs=1) as phc, \
                         tc.tile_pool(name=f"lps{ph}", bufs=4, space="PSUM") as phps:
                        cst = {}
                        for dr in ("f", "b"):
                            cst[dr] = phc.tile([H, BC], f32, tag=f"c{dr}", name=f"cst{dr}")
                            nc.vector.memset(cst[dr][:], 0.0)
                        halfs = phc.tile([H, 3 * BC], f32, tag="halfs")
                        nc.vector.memset(halfs[:], 0.5)
                        xw_b = {}
                        for t in range(T):
                            tt = T - 1 - t
                            if t % XWCH == 0:
                                for dr in ("f", "b"):
                                    # step-major: [:, s, :] is the contiguous
                                    # [H, 4*BC] gate row for step s
                                    xw_b[dr] = pxw.tile([H, XWCH, 4 * BC], bf16, tag=f"xw{dr}", name=f"xwb{dr}")
                                    base = t if dr == "f" else tt - XWCH + 1
                                    for Gb in range(4):
                                        nc.sync.dma_start(
                                            xw_b[dr][:, :, Gb*BC:(Gb+1)*BC],
                                            xwd[ph+dr].ap()[:, Gb*TB + base*BC: Gb*TB + (base+XWCH)*BC].rearrange(
                                                "h (s b) -> h s b", b=BC))
                            of = t % XWCH
                            ob = XWCH - 1 - (t % XWCH)
                            gate_ps, sact = {}, {}
                            for d_i, dr in ((0, "f"), (1, "b")):
                                st = hf_st if d_i == 0 else hb_st
                                tm = t if d_i == 0 else tt
                                if t == 0:
                                    hprev = zeroBC[:]
                                elif d_i == 0:
                                    hprev = st[:, (tm-1)*BC:tm*BC]
                                else:
                                    hprev = st[:, (tm+1)*BC:(tm+2)*BC]
                                wt = whh_sb[ph + dr]
                                gp_ = phps.tile([H, 4 * BC], f32, tag=f"g{dr}", name=f"gps{dr}")
                                gate_ps[dr] = gp_
                                off = of if d_i == 0 else ob
                                if USE_IDMM:
                                    nc.tensor.matmul(
                                        gp_[:], lhsT=ident_sb[:],
                                        rhs=xw_b[dr][:, off, :],
                                        start=True, stop=False)
                                    for G in range(4):
                                        blk = GBLK[G]
                                        nc.tensor.matmul(
                                            gp_[:, blk*BC:(blk+1)*BC],
                                            lhsT=wt[:, G*H:(G+1)*H], rhs=hprev,
                                            start=False, stop=True)
                                else:
                                    for G in range(4):
                                        blk = GBLK[G]
                                        nc.tensor.matmul(
                                            gp_[:, blk*BC:(blk+1)*BC],
                                            lhsT=wt[:, G*H:(G+1)*H], rhs=hprev,
                                            start=True, stop=True)
                            for dr in ("f", "b"):
                                sa = php.tile([H, 4 * BC], f32, tag=f"sa{dr}", name=f"sa{dr}")
                                sact[dr] = sa
                                if USE_IDMM:
                                    # gates fully accumulated in PSUM (hh + xw
                                    # via identity matmul): ACT reads PSUM
                                    nc.scalar.activation(
                                        sa[:, 0:3*BC], gate_ps[dr][:, 0:3*BC],
                                        AF.Sigmoid)
                                    nc.scalar.activation(
                                        sa[:, 3*BC:4*BC],
                                        gate_ps[dr][:, 3*BC:4*BC], AF.Tanh)
                                else:
                                    off = of if dr == "f" else ob
                                    gf_ = php.tile([H, 4 * BC], f32, tag=f"gf{dr}", name=f"gf{dr}")
                                    nc.vector.tensor_add(
                                        gf_[:], gate_ps[dr][:],
                                        xw_b[dr][:, off, :])
                                    nc.scalar.activation(sa[:, 0:3*BC], gf_[:, 0:3*BC],
                                                         AF.Sigmoid)
                                    nc.scalar.activation(sa[:, 3*BC:4*BC],
                                                         gf_[:, 3*BC:4*BC], AF.Tanh)
                            mm = {}
                            for dr in ("f", "b"):
                                sa = sact[dr]
                                mm[dr] = php.tile([H, BC], f32, tag=f"mm{dr}", name=f"mmt{dr}")
                                nc.vector.tensor_mul(mm[dr][:], sa[:, 0:BC], sa[:, 3*BC:4*BC])
                            eng = nc.gpsimd if USE_GPS else nc.vector
                            for dr in ("f", "b"):
                                eng.tensor_mul(cst[dr][:], cst[dr][:], sact[dr][:, BC:2*BC])
                            for dr in ("f", "b"):
                                eng.tensor_add(cst[dr][:], cst[dr][:], mm[dr][:])
                            tc_ = {}
                            for dr in ("f", "b"):
                                tc_[dr] = php.tile([H, BC], f32, tag=f"tc{dr}", name=f"tct{dr}")
                                nc.scalar.activation(tc_[dr][:], cst[dr][:], AF.Tanh)
                            nc.vector.tensor_mul(
                                hf_st[:, t*BC:(t+1)*BC], sact["f"][:, 2*BC:3*BC], tc_["f"][:])
                            nc.vector.tensor_mul(
                                hb_st[:, tt*BC:(tt+1)*BC], sact["b"][:, 2*BC:3*BC], tc_["b"][:])

                def chunks0(key):
                    wchunks, cbias = wih_sb[key]
                    ch = [(wchunks[:, u*4*H:(u+1)*4*H], tokAll[:, u*TB:(u+1)*TB])
                          for u in range(5)]
                    ch.append((cbias, ones1))
                    return ch

                def chunks_for(key, rA, rB):
                    a, bwt, cbias = wih_sb[key]
                    return [(a, rA), (bwt, rB), (cbias, ones1)]
                xw_precompute("0", chunks0("0f"), chunks0("0b"))
                lstm_phase("0", h0f, h0b)
                if dbg:
                    nc.sync.dma_start(h0d.ap()[:, 0:TB], h0f[:])
                    nc.sync.dma_start(h0d.ap()[:, TB:2*TB], h0b[:])
                xw_precompute("1", chunks_for("1f", h0f, h0b),
                              chunks_for("1b", h0f, h0b))
                lstm_phase("1", h1f, h1b)

                wfc_a = lp.tile([H, NCLS], bf16)
                nc.sync.dma_start(wfc_a[:], wfc[0:H])
                wfc_b = lp.tile([H, NCLS], bf16)
                nc.sync.dma_start(wfc_b[:], wfc[H:])
                bfc_sb = lp.tile([BC, NCLS], f32)
                nc.sync.dma_start(bfc_sb[:], bfc[:])
                with tc.tile_pool(name="fcps", bufs=1, space="PSUM") as fps:
                    fc_ps = fps.tile([BC, NCLS], f32)
                    nc.tensor.matmul(fc_ps[:], lhsT=h1f[:, (T-1)*BC:T*BC],
                                     rhs=wfc_a[:], start=True, stop=False)
                    nc.tensor.matmul(fc_ps[:], lhsT=h1b[:, (T-1)*BC:T*BC],
                                     rhs=wfc_b[:], start=False, stop=True)
                    fc_sb = lp.tile([BC, NCLS], f32)
                    nc.vector.tensor_add(fc_sb[:], fc_ps[:], bfc_sb[:])
                    nc.sync.dma_start(out[:], fc_sb[:])
    return nc


# =====================================================================
# Entry
# =====================================================================

_CACHED = {}

def kernel(**inputs):
    xg_cores, idx_cores, S_cores = _prep_graph(
        inputs["x"], inputs["edge_src"], inputs["edge_dst"])
    wts = _prep_weights(inputs)

    if "nc" not in _CACHED:
        nc_new = build_kernel()
        if not nc_new.is_finalized():
            nc_new.finalize()
        _CACHED["nc"] = nc_new
    nc = _CACHED["nc"]

    in_maps = []
    for c in range(NCORES):
        m = dict(
            xg=xg_cores[c], idxT2=idx_cores[c],
            ST=S_cores[c][0], SSP=S_cores[c][1],
            identT=np.eye(128, dtype=np.float32).astype(BF16),
            W1=wts["W1"], W2=wts["W2"], W3=wts["W3"],
            b1=wts["b1"], b2=wts["b2"], b3=wts["b3"],
            wfc=wts["wfc"], bfc=wts["bfc"],
        )
        for tag in ("0", "1"):
            for dr in ("f", "b"):
                m[f"wih{tag}{dr}"] = wts[f"wih{tag}{dr}"]
                m[f"whh{tag}{dr}"] = wts[f"whh{tag}{dr}"]
        in_maps.append(m)

    res = run_bass_kernel_spmd(nc, in_maps, core_ids=list(range(NCORES)),
                               trace=os.environ.get("KTRACE", "0") == "1")
    kernel.last_result = res
    kernel.last_debug = res.results if os.environ.get("KDEBUG") == "1" else None
    outs = [res.results[c]["out"] for c in range(NCORES)]
    return np.concatenate(outs, axis=0).astype(np.float32)

